# revision 24
# baseline (speedup 1.0000x reference)
"""Trainium2 Bass kernel for the CPCA auxiliary loss (nn_CPCA_51754355917033).

Strategy (data-parallel over the env/batch dim n, 16 envs per core):
  - Host side: every gather is baked into per-core contiguous device
    inputs (action-embedding -> gi with b_ih and the r/z part of b_hh
    folded in, h0, targets, negatives, forward mask).
  - GRU: r/z gate adds are plain TT adds (bias pre-folded); the g-gate
    bias is injected with a K=1 ones-matmul so the r* product reads raw
    PSUM; all post-PSUM element-wise work runs in bf16 SBUF (fast DVE
    modes).  The serial per-step tail would leave the PE idle ~4us per
    step (and drop it to the 1.2 GHz p-state): those gaps are filled
    with the L1 x@W1b matmuls of one MLP block per step ("hoisted"
    blocks), keeping the PE at full clock.
  - MLP: preds @ W1a computed once (pa); per block only x @ W1b (16
    MMs, absent for hoisted blocks), L2 (16 MMs), fp8-DoubleRow L3 with
    y2 stationary.  Evictions balanced across DVE and ACT.
  - Tail: batched softplus (relu - ln(sigmoid(|.|))) + accum_out fused
    masked sums.  Host combines the 8 cores' partials.
"""

import numpy as np
import ml_dtypes

import concourse.bass as bass
import concourse.mybir as mybir
import concourse.tile as tile
from concourse import bacc
from concourse import bass_utils

BF16 = ml_dtypes.bfloat16
F8 = ml_dtypes.float8_e4m3
DT = mybir.dt
AF = mybir.ActivationFunctionType
ALU = mybir.AluOpType
DRM = mybir.MatmulPerfMode.DoubleRow

N, T, H, K, S, F, EMB, NLOG, NEG = 128, 512, 512, 16, 16, 4, 32, 18, 20
COEFF = 0.1
NC = 8
NPC = N // NC          # envs per core
R = NPC * S            # GRU rows per core (256)
L = T - 1
NBLK = NEG + 1         # 20 negative g-blocks + 1 positive block
BR = F * R             # rows per block (1024)

_PROGRAM_CACHE = {}


# ----------------------------------------------------------------- host prep

def _prep_shared(inputs, u_list, k_eff):
    """Per-run (not per-core) preprocessing."""
    W_ih = np.asarray(inputs["W_ih"], np.float32)
    W_hh = np.asarray(inputs["W_hh"], np.float32)
    b_ih = np.asarray(inputs["b_ih"], np.float32)
    b_hh = np.asarray(inputs["b_hh"], np.float32)
    W1 = np.asarray(inputs["W1"], np.float32)
    b1 = np.asarray(inputs["b1"], np.float32)
    W2 = np.asarray(inputs["W2"], np.float32)
    b2 = np.asarray(inputs["b2"], np.float32)
    W3 = np.asarray(inputs["W3"], np.float32)
    b3 = np.asarray(inputs["b3"], np.float32)
    emb_tab = np.asarray(inputs["action_embed"], np.float32)

    d = {}
    # GIE: action -> 1536-dim gi with b_ih everywhere and the r/z part of
    # b_hh folded in (its g part sits inside the r* product).
    GIE = np.zeros((NLOG + 1, 1536), np.float32)
    GIE[:NLOG] = emb_tab @ W_ih.T
    GIE += b_ih
    GIE[:, :1024] += b_hh[:1024]
    d["GIE"] = GIE

    d["w_hh8"] = np.ascontiguousarray(
        W_hh.T.reshape(2, 2, 128, 1536).transpose(0, 2, 1, 3)).astype(F8)

    def pack8(WT):
        # [t, ki, ko, m] with contract index = t*256 + ko*128 + ki
        return np.ascontiguousarray(
            WT.reshape(2, 2, 128, WT.shape[1]).transpose(0, 2, 1, 3)).astype(F8)
    d["w1a8"] = pack8(W1[:, :512].T.copy())
    d["w1b8"] = pack8(W1[:, 512:].T.copy())
    d["w28"] = pack8(W2.T.copy())
    # w3 stationary for fp8-DR L3: [128, th, dr, 1], k = th*256 + dr*128 + p
    d["w38"] = np.ascontiguousarray(
        W3[0].reshape(2, 2, 128).transpose(2, 0, 1).reshape(128, 2, 2, 1)
    ).astype(F8)
    # b_hh g part for the K=1 bias matmul: [1, 4, 128] bf16
    d["bhhg"] = np.ascontiguousarray(
        b_hh[1024:].reshape(1, 4, 128)).astype(BF16)
    d["b1T"] = np.ascontiguousarray(b1.reshape(4, 128).T).astype(np.float32)
    d["b2T"] = np.ascontiguousarray(b2.reshape(4, 128).T).astype(np.float32)
    d["b3f"] = float(b3.reshape(-1)[0])

    ti = np.asarray(inputs["time_subsample"]).astype(np.int64)
    idx = np.arange(k_eff)[:, None] + ti[None, :]          # (k_eff, S)
    d["ti"] = ti
    d["idx"] = idx
    return d


def _prep_core(c, inputs, shared, u_list, k_eff):
    acts = np.asarray(inputs["actions"])[..., 0]
    nd = np.asarray(inputs["not_dones"])[..., 0]
    ri = np.asarray(inputs["rnn_inputs"], np.float32)
    ro = np.asarray(inputs["rnn_outputs"], np.float32)
    neg_idx = np.asarray(inputs["neg_idx"]).astype(np.int64)
    ti, idx = shared["ti"], shared["idx"]

    ns = slice(c * NPC, (c + 1) * NPC)

    # gi for all 12 gate chunks: [k_eff, 128, 12, R] bf16
    act_ext = np.full((NPC, L + K), NLOG, np.int64)
    act_ext[:, :L] = acts[ns, :L]
    AI = act_ext[:, idx].transpose(1, 0, 2).reshape(k_eff, R)   # (k_eff, R)
    gi_all = shared["GIE"][AI]                                  # (k_eff, R, 1536)
    giT = np.ascontiguousarray(
        gi_all.transpose(0, 2, 1).reshape(k_eff, 12, 128, R)
        .transpose(0, 2, 1, 3)).astype(BF16)                    # (k_eff,128,12,R)

    H0 = ro[ns][:, ti]                                          # (NPC, S, H)
    h0T = np.ascontiguousarray(
        H0.transpose(2, 0, 1).reshape(4, 128, R)).astype(BF16)

    ri_ext = np.zeros((NPC, L + K, H), np.float32)
    ri_ext[:, :L] = ri[ns, 1:]
    idx2 = np.asarray(u_list)[:, None] + ti[None, :]            # (F, S)
    TG = ri_ext[:, idx2]                                        # (NPC, F, S, H)
    tgT = np.ascontiguousarray(
        TG.transpose(3, 1, 0, 2).reshape(4, 128, BR)).astype(F8)

    ni = neg_idx.reshape(F, N, S, NEG)[:, ns]                   # (F, NPC, S, NEG)
    P = ni.transpose(3, 0, 1, 2).reshape(-1)                    # (g, f, n, s)
    negs = ri.reshape(N * T, H)[P]                              # (NEG*BR, H)
    negsT = np.ascontiguousarray(
        negs.T.reshape(4, 128, NEG, BR).transpose(2, 1, 0, 3)).astype(F8)

    # forward mask on host: mfT [128, 2F] f32, j = 2*fi + half,
    # row = half*128 + p  (rows are (n, s) flattened, 256 per core)
    nd_ext = np.zeros((NPC, L + K), np.float32)
    nd_ext[:, :L] = nd[ns, :L]
    G = (nd_ext[:, idx] > 0)                                    # (NPC, k_eff, S)
    fm = np.cumprod(G.transpose(1, 0, 2).reshape(k_eff, R), axis=0) > 0
    mf = fm[np.asarray(u_list)].astype(np.float32)              # (F, R)
    mfT = np.ascontiguousarray(
        mf.reshape(F, 2, 128).transpose(2, 0, 1).reshape(128, 2 * F)
    ).astype(np.float32)

    return dict(giT=giT, h0T=h0T, tgT=tgT, negsT=negsT, mfT=mfT)


# ------------------------------------------------------------- device program

def _build_program(u_list, k_eff, b3f):
    nc = bacc.Bacc("TRN2", target_bir_lowering=False, debug=False,
                   num_devices=NC)

    f32, bf16, f8 = DT.float32, DT.bfloat16, DT.float8e4

    def inp(name, shape, dt):
        return nc.dram_tensor(name, list(shape), dt, kind="ExternalInput")

    d_whh = inp("w_hh8", (2, 128, 2, 1536), f8)
    d_gi = inp("giT", (k_eff, 128, 12, R), bf16)
    d_h0 = inp("h0T", (4, 128, R), bf16)
    d_w1a = inp("w1a8", (2, 128, 2, 512), f8)
    d_w1b = inp("w1b8", (2, 128, 2, 512), f8)
    d_w2 = inp("w28", (2, 128, 2, 512), f8)
    d_w3 = inp("w38", (128, 2, 2, 1), f8)
    d_bhhg = inp("bhhg", (1, 4, 128), bf16)
    d_b1 = inp("b1T", (128, 4), f32)
    d_b2 = inp("b2T", (128, 4), f32)
    d_tg = inp("tgT", (4, 128, BR), f8)
    d_negs = inp("negsT", (NEG, 128, 4, BR), f8)
    d_mf = inp("mfT", (128, 2 * F), f32)
    d_out = nc.dram_tensor("out", [1, 4], f32, kind="ExternalOutput")

    with tile.TileContext(nc) as tc:
        with (
            tc.tile_pool(name="const", bufs=1) as cp,
            tc.tile_pool(name="gru", bufs=2) as gp,
            tc.tile_pool(name="mlp", bufs=2) as mp,
            tc.tile_pool(name="ps", bufs=3, space="PSUM") as pm,
            tc.tile_pool(name="psa", bufs=2, space="PSUM") as pa_pool,
        ):
            # ---------------------------------------------- constant loads
            # startup-critical first: whh + h0 + bias + gi0 + w1b
            whh = cp.tile([128, 2, 2, 1536], f8, tag="whh")
            for th in range(2):
                nc.sync.dma_start(out=whh[:, th, :, :], in_=d_whh[th])
            h_prev = gp.tile([128, 4, R], bf16, tag="h")
            for kc in range(4):
                nc.sync.dma_start(out=h_prev[:, kc, :], in_=d_h0[kc])
            bhhg = cp.tile([1, 4, 128], bf16, tag="bhhg")
            nc.sync.dma_start(out=bhhg[:], in_=d_bhhg[:])
            ones1 = cp.tile([1, R], bf16, tag="ones1")
            nc.any.memset(ones1[:], 1.0)
            gi_tiles = []
            for k in range(k_eff):
                gt = gp.tile([128, 12, R], bf16, tag="gi", bufs=3)
                gi_tiles.append(gt)
                if k == 0:
                    nc.sync.dma_start(out=gt[:], in_=d_gi[0])
            w1b = cp.tile([128, 2, 2, 512], f8, tag="w1b")
            for th in range(2):
                nc.sync.dma_start(out=w1b[:, th, :, :], in_=d_w1b[th])
            tg = cp.tile([128, 4, BR], f8, tag="tg")
            for kc in range(4):
                nc.sync.dma_start(out=tg[:, kc, :], in_=d_tg[kc])

            # rest of the constants (needed only for pa / MLP phase)
            w1a = cp.tile([128, 2, 2, 512], f8, tag="w1a")
            w2 = cp.tile([128, 2, 2, 512], f8, tag="w2")
            for (t, dd) in ((w1a, d_w1a), (w2, d_w2)):
                for th in range(2):
                    nc.sync.dma_start(out=t[:, th, :, :], in_=dd[th])
            w38 = cp.tile([128, 2, 2, 1], f8, tag="w38")
            nc.sync.dma_start(out=w38[:], in_=d_w3[:])
            b1 = cp.tile([128, 4], f32, tag="b1")
            nc.sync.dma_start(out=b1[:], in_=d_b1[:])
            b2 = cp.tile([128, 4], f32, tag="b2")
            nc.sync.dma_start(out=b2[:], in_=d_b2[:])
            mfT = cp.tile([128, 2 * F], f32, tag="mfT")
            nc.sync.dma_start(out=mfT[:], in_=d_mf[:])

            # ------------------------------------------------ GRU
            h8_prev = gp.tile([128, 4, R], f8, tag="h8")
            nc.vector.tensor_copy(h8_prev[:], h_prev[:])
            predsT = cp.tile([128, 4, BR], f8, tag="preds")

            def l1_matmuls(ps, cc, xt):
                for th in range(2):
                    for rt in range(2):
                        nc.tensor.matmul(
                            ps[:, 2 * rt:2 * rt + 2, :],
                            w1b[:, th, :, cc * 128:(cc + 1) * 128],
                            xt[:, 2 * th:2 * th + 2,
                               rt * 512:(rt + 1) * 512],
                            start=(th == 0), stop=(th == 1),
                            perf_mode=DRM, skip_group_check=True)

            # manual schedule ladder: the greedy scheduler (whose sim
            # under-models the hardware) otherwise reorders within engine
            # queues in ways that stretch the serial GRU tail.
            STEP_MS = 0.02
            DUMMIES = 8

            for k in range(k_eff):
                gi = gi_tiles[k]
                tc.tile_set_cur_wait(k * STEP_MS + 0.001)
                if k + 1 < k_eff:
                    nc.sync.dma_start(out=gi_tiles[k + 1][:], in_=d_gi[k + 1])

                psr = pm.tile([128, 4, R], f32, tag="pm")
                psz = pm.tile([128, 4, R], f32, tag="pm")
                psg = pm.tile([128, 4, R], f32, tag="pm")
                rp_sb = gp.tile([128, 4, R], bf16, tag="rp", bufs=1)
                zp_sb = gp.tile([128, 4, R], bf16, tag="zp", bufs=1)
                r_sb = gp.tile([128, 4, R], bf16, tag="r", bufs=1)
                z_sb = gp.tile([128, 4, R], bf16, tag="z", bufs=1)
                t_sb = gp.tile([128, 4, R], bf16, tag="t", bufs=1)
                u_sb = gp.tile([128, 4, R], bf16, tag="u", bufs=1)
                g_sb = gp.tile([128, 4, R], bf16, tag="g", bufs=1)
                e_sb = gp.tile([128, 4, R], bf16, tag="e", bufs=1)
                w1m = gp.tile([128, 4, R], bf16, tag="w1m", bufs=1)
                gw = gp.tile([128, 4, R], bf16, tag="gw", bufs=1)
                h8_new = gp.tile([128, 4, R], f8, tag="h8")

                # tensor queue: r(8), z(8), bias-g(4)+g(8), then dummy fill
                for pst, base in ((psr, 0), (psz, 4)):
                    for c in range(4):
                        gc = base + c
                        for th in range(2):
                            nc.tensor.matmul(
                                pst[:, c, :],
                                whh[:, th, :, gc * 128:(gc + 1) * 128],
                                h8_prev[:, 2 * th:2 * th + 2, :],
                                start=(th == 0), stop=(th == 1),
                                perf_mode=DRM, skip_group_check=True)
                for c in range(4):
                    nc.tensor.matmul(
                        psg[:, c, :], bhhg[:, c, :], ones1[:],
                        start=True, stop=False, skip_group_check=True)
                    gc = 8 + c
                    for th in range(2):
                        nc.tensor.matmul(
                            psg[:, c, :],
                            whh[:, th, :, gc * 128:(gc + 1) * 128],
                            h8_prev[:, 2 * th:2 * th + 2, :],
                            start=False, stop=(th == 1), perf_mode=DRM,
                            skip_group_check=True)

                # DVE: rz adds in bf16, then the serial tail (h kept in
                # f8; t/u/tanh split in kc halves to overlap ACT and DVE)
                nc.vector.tensor_add(rp_sb[:], psr[:], gi[:, 0:4, :])
                nc.vector.tensor_add(zp_sb[:], psz[:], gi[:, 4:8, :])
                nc.scalar.activation(r_sb[:], rp_sb[:], AF.Sigmoid)
                nc.scalar.activation(z_sb[:], zp_sb[:], AF.Sigmoid)
                for hh in range(2):
                    s = slice(2 * hh, 2 * hh + 2)
                    nc.vector.tensor_mul(t_sb[:, s, :], psg[:, s, :],
                                         r_sb[:, s, :])
                    nc.vector.tensor_add(u_sb[:, s, :], t_sb[:, s, :],
                                         gi[:, 8 + 2 * hh:10 + 2 * hh, :])
                    nc.scalar.activation(g_sb[:, s, :], u_sb[:, s, :],
                                         AF.Tanh)
                nc.gpsimd.tensor_scalar(w1m[:], z_sb[:], -1.0, 1.0,
                                        op0=ALU.mult, op1=ALU.add)
                nc.vector.tensor_mul(e_sb[:], z_sb[:], h8_prev[:])
                nc.vector.tensor_mul(gw[:], g_sb[:], w1m[:])
                nc.vector.tensor_add(h8_new[:], gw[:], e_sb[:])

                # dummy matmuls (outputs never read): keep the PE busy
                # through the serial tail so the p-state governor holds the
                # full clock; at half clock every real matmul costs ~1.6x.
                with tc.tile_wait_until(k * STEP_MS + 0.010):
                    psd = pm.tile([128, 4, R], f32, tag="pm")
                    for dmy in range(DUMMIES):
                        nc.tensor.matmul(
                            psd[:, 2 * (dmy % 2):2 * (dmy % 2) + 2, :],
                            w1b[:, 0, :, 0:128],
                            tg[:, 0:2, 0:512],
                            start=True, stop=True, perf_mode=DRM,
                            skip_group_check=True)

                h8_prev = h8_new
                for fi, u in enumerate(u_list):
                    if u == k:
                        with tc.tile_wait_until(k * STEP_MS + 0.016):
                            nc.scalar.activation(
                                predsT[:, :, fi * R:(fi + 1) * R],
                                h8_new[:], AF.Copy)

            # ------------------------------------------------ pa = preds@W1a
            MLP0 = k_eff * STEP_MS + 0.005
            BLK_MS = 0.012
            tc.tile_set_cur_wait(MLP0)
            pa_sb = cp.tile([128, 4, BR], bf16, tag="pa")
            for cc in range(4):
                ps = pm.tile([128, 4, R], f32, tag="pm")
                for th in range(2):
                    for rt in range(2):
                        nc.tensor.matmul(
                            ps[:, 2 * rt:2 * rt + 2, :],
                            w1a[:, th, :, cc * 128:(cc + 1) * 128],
                            predsT[:, 2 * th:2 * th + 2,
                                   rt * 512:(rt + 1) * 512],
                            start=(th == 0), stop=(th == 1), perf_mode=DRM,
                            skip_group_check=True)
                nc.vector.tensor_scalar(pa_sb[:, cc, :], ps[:],
                                        b1[:, cc:cc + 1], None, op0=ALU.add)

            # ------------------------------------------------ blocks
            # 3-stage software pipeline: window w runs L1(w), L2(w-1) and
            # L3(w-2).  The one-window delay between a stage's producers and
            # consumers means no tensor-queue instruction ever waits on an
            # eviction — the PE stays saturated and holds the full clock.
            logits = cp.tile([128, NBLK, 8], f32, tag="logits")
            y1_t, y2_t, ps3_t = {}, {}, {}
            for w in range(NBLK + 2):
                tc.tile_set_cur_wait(MLP0 + (w + 1) * BLK_MS)
                if w < NBLK:
                    b = w
                    if b < NEG:
                        xt = mp.tile([128, 4, BR], f8, tag="negsx", bufs=3)
                        nc.sync.dma_start(out=xt[:], in_=d_negs[b])
                    else:
                        xt = tg
                    y1p = mp.tile([128, 4, BR], bf16, tag="y1p", bufs=2)
                    y1 = mp.tile([128, 4, BR], f8, tag="y1", bufs=3)
                    y1_t[b] = y1
                    for cc in range(4):
                        ps = pm.tile([128, 4, R], f32, tag="pm")
                        l1_matmuls(ps, cc, xt)
                        nc.vector.tensor_add(y1p[:, cc, :], ps[:],
                                             pa_sb[:, cc, :])
                        nc.gpsimd.tensor_scalar(y1[:, cc, :], y1p[:, cc, :],
                                                0.0, None, op0=ALU.max)
                if 0 <= w - 1 < NBLK:
                    b = w - 1
                    y1 = y1_t.pop(b)
                    y2 = mp.tile([128, 4, BR], f8, tag="y2", bufs=3)
                    y2_t[b] = y2
                    with tc.tile_wait_until(MLP0 + (w + 1) * BLK_MS + 0.004):
                        for cc in range(4):
                            ps = pm.tile([128, 4, R], f32, tag="pm")
                            for th in range(2):
                                for rt in range(2):
                                    nc.tensor.matmul(
                                        ps[:, 2 * rt:2 * rt + 2, :],
                                        w2[:, th, :, cc * 128:(cc + 1) * 128],
                                        y1[:, 2 * th:2 * th + 2,
                                           rt * 512:(rt + 1) * 512],
                                        start=(th == 0), stop=(th == 1),
                                        perf_mode=DRM, skip_group_check=True)
                            nc.scalar.activation(y2[:, cc, :], ps[:], AF.Relu,
                                                 bias=b2[:, cc:cc + 1])
                if 0 <= w - 2 < NBLK:
                    b = w - 2
                    y2 = y2_t.pop(b)
                    ps3 = pa_pool.tile([128, 8], f32, tag="pa3")
                    with tc.tile_wait_until(MLP0 + (w + 1) * BLK_MS + 0.008):
                        for col in range(8):
                            for th in range(2):
                                nc.tensor.matmul(
                                    ps3[:, col:col + 1],
                                    y2[:, 2 * th:2 * th + 2,
                                       col * 128:(col + 1) * 128],
                                    w38[:, th, :, :],
                                    start=(th == 0), stop=(th == 1),
                                    perf_mode=DRM, skip_group_check=True)
                        nc.scalar.activation(logits[:, b, :], ps3[:], AF.Copy)

            # ------------------------------------- softplus + masked sums
            # softplus(t) = relu(t) - ln(sigmoid(|t|)); |t| is sign-agnostic
            # so Abs/Sigmoid/Ln run over all 21 blocks at once.
            tc.tile_set_cur_wait(MLP0 + (NBLK + 4) * BLK_MS)
            sp_a = cp.tile([128, NBLK, 8], f32, tag="sp_a")
            sp_r = cp.tile([128, NBLK, 8], f32, tag="sp_r")
            sp = cp.tile([128, NBLK, 8], f32, tag="sp")
            spn_sum = cp.tile([128, 8], f32, tag="spn_sum")
            junk = cp.tile([128, 3, 8], f32, tag="junk")
            vcol = cp.tile([128, 4], f32, tag="vcol")
            nc.scalar.activation(sp_a[:], logits[:], AF.Abs, bias=b3f)
            nc.scalar.activation(sp_r[:, :NEG, :], logits[:, :NEG, :],
                                 AF.Relu, bias=b3f)
            nc.scalar.activation(sp_r[:, NEG, :], logits[:, NEG, :],
                                 AF.Relu, bias=-b3f, scale=-1.0)
            nc.scalar.activation(sp_a[:], sp_a[:], AF.Sigmoid)
            nc.scalar.activation(sp_a[:], sp_a[:], AF.Ln)
            nc.vector.tensor_sub(sp[:], sp_r[:], sp_a[:])
            nc.vector.tensor_reduce(
                spn_sum[:], sp[:, :NEG, :].transpose([0, 2, 1]),
                mybir.AxisListType.X, ALU.add)
            nc.any.memset(vcol[:], 0.0)
            nc.vector.scalar_tensor_tensor(
                junk[:, 0, :], in0=sp[:, NEG, :], scalar=1.0, in1=mfT[:],
                op0=ALU.mult, op1=ALU.mult, accum_out=vcol[:, 0:1])
            nc.vector.scalar_tensor_tensor(
                junk[:, 1, :], in0=spn_sum[:], scalar=1.0, in1=mfT[:],
                op0=ALU.mult, op1=ALU.mult, accum_out=vcol[:, 1:2])
            nc.vector.tensor_scalar(
                junk[:, 2, :], mfT[:], 1.0, 0.0, op0=ALU.mult,
                op1=ALU.add, accum_out=vcol[:, 2:3])

            ones = cp.tile([128, 1], f32, tag="ones")
            nc.any.memset(ones[:], 1.0)
            psf = pa_pool.tile([128, 8], f32, tag="pa3")
            nc.tensor.matmul(psf[0:1, 0:4], ones[:], vcol[:],
                             start=True, stop=True)
            out_sb = cp.tile([1, 4], f32, tag="out_sb")
            nc.scalar.activation(out_sb[:], psf[0:1, 0:4], AF.Copy)
            nc.sync.dma_start(out=d_out[:], in_=out_sb[:])

    nc.finalize()
    return nc


def _get_program(u_list, k_eff, b3f):
    key = (tuple(u_list), k_eff, float(b3f))
    if key not in _PROGRAM_CACHE:
        _PROGRAM_CACHE[key] = _build_program(u_list, k_eff, b3f)
    return _PROGRAM_CACHE[key]


# ------------------------------------------------------------------ kernel

def kernel(**inputs):
    u_list = [int(x) for x in np.asarray(inputs["unroll_subsample"]).reshape(-1)]
    k_eff = max(u_list) + 1
    shared = _prep_shared(inputs, u_list, k_eff)
    nc = _get_program(u_list, k_eff, shared["b3f"])

    wmaps = {k: v for k, v in shared.items()
             if k in ("w_hh8", "w1a8", "w1b8", "w28", "w38", "bhhg",
                      "b1T", "b2T")}
    in_maps = []
    for c in range(NC):
        m = dict(wmaps)
        m.update(_prep_core(c, inputs, shared, u_list, k_eff))
        in_maps.append(m)

    res = bass_utils.run_bass_kernel_spmd(nc, in_maps, list(range(NC)))
    P = Ng = D = 0.0
    for c in range(NC):
        o = np.asarray(res.results[c]["out"], np.float64)
        P += o[0, 0]
        Ng += o[0, 1]
        D += o[0, 2]
    loss = COEFF * (P / D + Ng / (D * NEG))
    return np.float32(loss)


# revision 25
# speedup vs baseline: 3.5609x; 3.5609x over previous
"""Trainium2 Bass kernel for the CPCA auxiliary loss (nn_CPCA_51754355917033).

Strategy (data-parallel over the env/batch dim n, 16 envs per core):
  - Host side: every gather is baked into per-core contiguous device
    inputs (action-embedding -> gi with b_ih and the r/z part of b_hh
    folded in, h0, targets, negatives, forward mask).
  - GRU: r/z gate adds are plain TT adds (bias pre-folded); the g-gate
    bias is injected with a K=1 ones-matmul so the r* product reads raw
    PSUM; all post-PSUM element-wise work runs in bf16 SBUF (fast DVE
    modes).  The serial per-step tail would leave the PE idle ~4us per
    step (and drop it to the 1.2 GHz p-state): those gaps are filled
    with the L1 x@W1b matmuls of one MLP block per step ("hoisted"
    blocks), keeping the PE at full clock.
  - MLP: preds @ W1a computed once (pa); per block only x @ W1b (16
    MMs, absent for hoisted blocks), L2 (16 MMs), fp8-DoubleRow L3 with
    y2 stationary.  Evictions balanced across DVE and ACT.
  - Tail: batched softplus (relu - ln(sigmoid(|.|))) + accum_out fused
    masked sums.  Host combines the 8 cores' partials.
"""

import numpy as np
import ml_dtypes

import concourse.bass as bass
import concourse.mybir as mybir
import concourse.tile as tile
from concourse import bacc
from concourse import bass_utils

BF16 = ml_dtypes.bfloat16
F8 = ml_dtypes.float8_e4m3
DT = mybir.dt
AF = mybir.ActivationFunctionType
ALU = mybir.AluOpType
DRM = mybir.MatmulPerfMode.DoubleRow

N, T, H, K, S, F, EMB, NLOG, NEG = 128, 512, 512, 16, 16, 4, 32, 18, 20
COEFF = 0.1
NC = 8
NPC = N // NC          # envs per core
R = NPC * S            # GRU rows per core (256)
L = T - 1
NBLK = NEG + 1         # 20 negative g-blocks + 1 positive block
BR = F * R             # rows per block (1024)

_PROGRAM_CACHE = {}


# ----------------------------------------------------------------- host prep

def _prep_shared(inputs, u_list, k_eff):
    """Per-run (not per-core) preprocessing."""
    W_ih = np.asarray(inputs["W_ih"], np.float32)
    W_hh = np.asarray(inputs["W_hh"], np.float32)
    b_ih = np.asarray(inputs["b_ih"], np.float32)
    b_hh = np.asarray(inputs["b_hh"], np.float32)
    W1 = np.asarray(inputs["W1"], np.float32)
    b1 = np.asarray(inputs["b1"], np.float32)
    W2 = np.asarray(inputs["W2"], np.float32)
    b2 = np.asarray(inputs["b2"], np.float32)
    W3 = np.asarray(inputs["W3"], np.float32)
    b3 = np.asarray(inputs["b3"], np.float32)
    emb_tab = np.asarray(inputs["action_embed"], np.float32)

    d = {}
    # GIE: action -> 1536-dim gi with b_ih everywhere and the r/z part of
    # b_hh folded in (its g part sits inside the r* product).
    GIE = np.zeros((NLOG + 1, 1536), np.float32)
    GIE[:NLOG] = emb_tab @ W_ih.T
    GIE += b_ih
    GIE[:, :1024] += b_hh[:1024]
    d["GIE"] = GIE

    d["w_hh8"] = np.ascontiguousarray(
        W_hh.T.reshape(2, 2, 128, 1536).transpose(0, 2, 1, 3)).astype(F8)

    def pack8(WT):
        # [t, ki, ko, m] with contract index = t*256 + ko*128 + ki
        return np.ascontiguousarray(
            WT.reshape(2, 2, 128, WT.shape[1]).transpose(0, 2, 1, 3)).astype(F8)
    d["w1a8"] = pack8(W1[:, :512].T.copy())
    d["w1b8"] = pack8(W1[:, 512:].T.copy())
    d["w28"] = pack8(W2.T.copy())
    # w3 stationary for fp8-DR L3: [128, th, dr, 1], k = th*256 + dr*128 + p
    d["w38"] = np.ascontiguousarray(
        W3[0].reshape(2, 2, 128).transpose(2, 0, 1).reshape(128, 2, 2, 1)
    ).astype(F8)
    # b_hh g part for the K=1 bias matmul: [1, 4, 128] bf16
    d["bhhg"] = np.ascontiguousarray(
        b_hh[1024:].reshape(1, 4, 128)).astype(BF16)
    d["b1T"] = np.ascontiguousarray(b1.reshape(4, 128).T).astype(np.float32)
    d["b2T"] = np.ascontiguousarray(b2.reshape(4, 128).T).astype(np.float32)
    d["b3f"] = float(b3.reshape(-1)[0])

    ti = np.asarray(inputs["time_subsample"]).astype(np.int64)
    idx = np.arange(k_eff)[:, None] + ti[None, :]          # (k_eff, S)
    d["ti"] = ti
    d["idx"] = idx
    return d


def _prep_core(c, inputs, shared, u_list, k_eff):
    acts = np.asarray(inputs["actions"])[..., 0]
    nd = np.asarray(inputs["not_dones"])[..., 0]
    ri = np.asarray(inputs["rnn_inputs"], np.float32)
    ro = np.asarray(inputs["rnn_outputs"], np.float32)
    neg_idx = np.asarray(inputs["neg_idx"]).astype(np.int64)
    ti, idx = shared["ti"], shared["idx"]

    ns = slice(c * NPC, (c + 1) * NPC)

    # gi for all 12 gate chunks: [k_eff, 128, 12, R] bf16
    act_ext = np.full((NPC, L + K), NLOG, np.int64)
    act_ext[:, :L] = acts[ns, :L]
    AI = act_ext[:, idx].transpose(1, 0, 2).reshape(k_eff, R)   # (k_eff, R)
    gi_all = shared["GIE"][AI]                                  # (k_eff, R, 1536)
    giT = np.ascontiguousarray(
        gi_all.transpose(0, 2, 1).reshape(k_eff, 12, 128, R)
        .transpose(0, 2, 1, 3)).astype(BF16)                    # (k_eff,128,12,R)

    H0 = ro[ns][:, ti]                                          # (NPC, S, H)
    h0T = np.ascontiguousarray(
        H0.transpose(2, 0, 1).reshape(4, 128, R)).astype(BF16)

    ri_ext = np.zeros((NPC, L + K, H), np.float32)
    ri_ext[:, :L] = ri[ns, 1:]
    idx2 = np.asarray(u_list)[:, None] + ti[None, :]            # (F, S)
    TG = ri_ext[:, idx2]                                        # (NPC, F, S, H)
    tgT = np.ascontiguousarray(
        TG.transpose(3, 1, 0, 2).reshape(4, 128, BR)).astype(F8)

    ni = neg_idx.reshape(F, N, S, NEG)[:, ns]                   # (F, NPC, S, NEG)
    P = ni.transpose(3, 0, 1, 2).reshape(-1)                    # (g, f, n, s)
    negs = ri.reshape(N * T, H)[P]                              # (NEG*BR, H)
    negsT = np.ascontiguousarray(
        negs.T.reshape(4, 128, NEG, BR).transpose(2, 1, 0, 3)).astype(F8)

    # forward mask on host: mfT [128, 2F] f32, j = 2*fi + half,
    # row = half*128 + p  (rows are (n, s) flattened, 256 per core)
    nd_ext = np.zeros((NPC, L + K), np.float32)
    nd_ext[:, :L] = nd[ns, :L]
    G = (nd_ext[:, idx] > 0)                                    # (NPC, k_eff, S)
    fm = np.cumprod(G.transpose(1, 0, 2).reshape(k_eff, R), axis=0) > 0
    mf = fm[np.asarray(u_list)].astype(np.float32)              # (F, R)
    mfT = np.ascontiguousarray(
        mf.reshape(F, 2, 128).transpose(2, 0, 1).reshape(128, 2 * F)
    ).astype(np.float32)

    return dict(giT=giT, h0T=h0T, tgT=tgT, negsT=negsT, mfT=mfT)


# ------------------------------------------------------------- device program

def _build_program(u_list, k_eff, b3f):
    nc = bacc.Bacc("TRN2", target_bir_lowering=False, debug=False,
                   num_devices=NC)

    f32, bf16, f8 = DT.float32, DT.bfloat16, DT.float8e4

    def inp(name, shape, dt):
        return nc.dram_tensor(name, list(shape), dt, kind="ExternalInput")

    d_whh = inp("w_hh8", (2, 128, 2, 1536), f8)
    d_gi = inp("giT", (k_eff, 128, 12, R), bf16)
    d_h0 = inp("h0T", (4, 128, R), bf16)
    d_w1a = inp("w1a8", (2, 128, 2, 512), f8)
    d_w1b = inp("w1b8", (2, 128, 2, 512), f8)
    d_w2 = inp("w28", (2, 128, 2, 512), f8)
    d_w3 = inp("w38", (128, 2, 2, 1), f8)
    d_bhhg = inp("bhhg", (1, 4, 128), bf16)
    d_b1 = inp("b1T", (128, 4), f32)
    d_b2 = inp("b2T", (128, 4), f32)
    d_tg = inp("tgT", (4, 128, BR), f8)
    d_negs = inp("negsT", (NEG, 128, 4, BR), f8)
    d_mf = inp("mfT", (128, 2 * F), f32)
    d_out = nc.dram_tensor("out", [1, 4], f32, kind="ExternalOutput")

    with tile.TileContext(nc) as tc:
        with (
            tc.tile_pool(name="const", bufs=1) as cp,
            tc.tile_pool(name="gru", bufs=2) as gp,
            tc.tile_pool(name="mlp", bufs=2) as mp,
            tc.tile_pool(name="ps", bufs=3, space="PSUM") as pm,
            tc.tile_pool(name="psa", bufs=2, space="PSUM") as pa_pool,
        ):
            # ---------------------------------------------- constant loads
            # startup-critical first: whh + h0 + bias + gi0 + w1b
            whh = cp.tile([128, 2, 2, 1536], f8, tag="whh")
            for th in range(2):
                nc.sync.dma_start(out=whh[:, th, :, :], in_=d_whh[th])
            h_prev = gp.tile([128, 4, R], bf16, tag="h")
            for kc in range(4):
                nc.sync.dma_start(out=h_prev[:, kc, :], in_=d_h0[kc])
            bhhg = cp.tile([1, 4, 128], bf16, tag="bhhg")
            nc.sync.dma_start(out=bhhg[:], in_=d_bhhg[:])
            ones1 = cp.tile([1, R], bf16, tag="ones1")
            nc.any.memset(ones1[:], 1.0)
            gi_tiles = []
            for k in range(k_eff):
                gt = gp.tile([128, 12, R], bf16, tag="gi", bufs=3)
                gi_tiles.append(gt)
                if k == 0:
                    nc.sync.dma_start(out=gt[:], in_=d_gi[0])
            w1b = cp.tile([128, 2, 2, 512], f8, tag="w1b")
            for th in range(2):
                nc.sync.dma_start(out=w1b[:, th, :, :], in_=d_w1b[th])
            tg = cp.tile([128, 4, BR], f8, tag="tg")
            for kc in range(4):
                nc.sync.dma_start(out=tg[:, kc, :], in_=d_tg[kc])

            # rest of the constants (needed only for pa / MLP phase)
            w1a = cp.tile([128, 2, 2, 512], f8, tag="w1a")
            w2 = cp.tile([128, 2, 2, 512], f8, tag="w2")
            for (t, dd) in ((w1a, d_w1a), (w2, d_w2)):
                for th in range(2):
                    nc.sync.dma_start(out=t[:, th, :, :], in_=dd[th])
            w38 = cp.tile([128, 2, 2, 1], f8, tag="w38")
            nc.sync.dma_start(out=w38[:], in_=d_w3[:])
            b1 = cp.tile([128, 4], f32, tag="b1")
            nc.sync.dma_start(out=b1[:], in_=d_b1[:])
            b2 = cp.tile([128, 4], f32, tag="b2")
            nc.sync.dma_start(out=b2[:], in_=d_b2[:])
            mfT = cp.tile([128, 2 * F], f32, tag="mfT")
            nc.sync.dma_start(out=mfT[:], in_=d_mf[:])

            # ------------------------------------------------ GRU
            h8_prev = gp.tile([128, 4, R], f8, tag="h8")
            nc.vector.tensor_copy(h8_prev[:], h_prev[:])
            predsT = cp.tile([128, 4, BR], f8, tag="preds")

            def l1_matmuls(ps, cc, xt):
                for th in range(2):
                    for rt in range(2):
                        nc.tensor.matmul(
                            ps[:, 2 * rt:2 * rt + 2, :],
                            w1b[:, th, :, cc * 128:(cc + 1) * 128],
                            xt[:, 2 * th:2 * th + 2,
                               rt * 512:(rt + 1) * 512],
                            start=(th == 0), stop=(th == 1),
                            perf_mode=DRM, skip_group_check=True)

            # manual schedule ladder: the greedy scheduler (whose sim
            # under-models the hardware) otherwise reorders within engine
            # queues in ways that stretch the serial GRU tail.
            STEP_MS = 0.02
            DUMMIES = 8

            for k in range(k_eff):
                gi = gi_tiles[k]
                tc.tile_set_cur_wait(k * STEP_MS + 0.001)
                if k + 1 < k_eff:
                    nc.sync.dma_start(out=gi_tiles[k + 1][:], in_=d_gi[k + 1])

                psr = pm.tile([128, 4, R], f32, tag="pm")
                psz = pm.tile([128, 4, R], f32, tag="pm")
                psg = pm.tile([128, 4, R], f32, tag="pm")
                rp_sb = gp.tile([128, 4, R], bf16, tag="rp", bufs=1)
                zp_sb = gp.tile([128, 4, R], bf16, tag="zp", bufs=1)
                r_sb = gp.tile([128, 4, R], bf16, tag="r", bufs=1)
                z_sb = gp.tile([128, 4, R], bf16, tag="z", bufs=1)
                t_sb = gp.tile([128, 4, R], bf16, tag="t", bufs=1)
                u_sb = gp.tile([128, 4, R], bf16, tag="u", bufs=1)
                g_sb = gp.tile([128, 4, R], bf16, tag="g", bufs=1)
                e_sb = gp.tile([128, 4, R], bf16, tag="e", bufs=1)
                w1m = gp.tile([128, 4, R], bf16, tag="w1m", bufs=1)
                gw = gp.tile([128, 4, R], bf16, tag="gw", bufs=1)
                h8_new = gp.tile([128, 4, R], f8, tag="h8")

                # tensor queue: r(8), z(8), bias-g(4)+g(8), then dummy fill
                for pst, base in ((psr, 0), (psz, 4)):
                    for c in range(4):
                        gc = base + c
                        for th in range(2):
                            nc.tensor.matmul(
                                pst[:, c, :],
                                whh[:, th, :, gc * 128:(gc + 1) * 128],
                                h8_prev[:, 2 * th:2 * th + 2, :],
                                start=(th == 0), stop=(th == 1),
                                perf_mode=DRM, skip_group_check=True)
                for c in range(4):
                    nc.tensor.matmul(
                        psg[:, c, :], bhhg[:, c, :], ones1[:],
                        start=True, stop=False, skip_group_check=True)
                    gc = 8 + c
                    for th in range(2):
                        nc.tensor.matmul(
                            psg[:, c, :],
                            whh[:, th, :, gc * 128:(gc + 1) * 128],
                            h8_prev[:, 2 * th:2 * th + 2, :],
                            start=False, stop=(th == 1), perf_mode=DRM,
                            skip_group_check=True)

                # DVE: rz adds in bf16, then the serial tail (h kept in
                # f8; t/u/tanh split in kc halves to overlap ACT and DVE)
                nc.vector.tensor_add(rp_sb[:], psr[:], gi[:, 0:4, :])
                nc.vector.tensor_add(zp_sb[:], psz[:], gi[:, 4:8, :])
                nc.scalar.activation(r_sb[:], rp_sb[:], AF.Sigmoid)
                nc.scalar.activation(z_sb[:], zp_sb[:], AF.Sigmoid)
                for hh in range(2):
                    s = slice(2 * hh, 2 * hh + 2)
                    nc.vector.tensor_mul(t_sb[:, s, :], psg[:, s, :],
                                         r_sb[:, s, :])
                    nc.vector.tensor_add(u_sb[:, s, :], t_sb[:, s, :],
                                         gi[:, 8 + 2 * hh:10 + 2 * hh, :])
                    nc.scalar.activation(g_sb[:, s, :], u_sb[:, s, :],
                                         AF.Tanh)
                nc.gpsimd.tensor_scalar(w1m[:], z_sb[:], -1.0, 1.0,
                                        op0=ALU.mult, op1=ALU.add)
                nc.vector.tensor_mul(e_sb[:], z_sb[:], h8_prev[:])
                nc.vector.tensor_mul(gw[:], g_sb[:], w1m[:])
                nc.vector.tensor_add(h8_new[:], gw[:], e_sb[:])

                # dummy matmuls (outputs never read): keep the PE busy
                # through the serial tail so the p-state governor holds the
                # full clock; at half clock every real matmul costs ~1.6x.
                with tc.tile_wait_until(k * STEP_MS + 0.010):
                    psd = pm.tile([128, 4, R], f32, tag="pm")
                    for dmy in range(DUMMIES):
                        nc.tensor.matmul(
                            psd[:, 2 * (dmy % 2):2 * (dmy % 2) + 2, :],
                            w1b[:, 0, :, 0:128],
                            tg[:, 0:2, 0:512],
                            start=True, stop=True, perf_mode=DRM,
                            skip_group_check=True)

                h8_prev = h8_new
                for fi, u in enumerate(u_list):
                    if u == k:
                        with tc.tile_wait_until(k * STEP_MS + 0.016):
                            nc.scalar.activation(
                                predsT[:, :, fi * R:(fi + 1) * R],
                                h8_new[:], AF.Copy)

            # ------------------------------------------------ pa = preds@W1a
            MLP0 = k_eff * STEP_MS + 0.005
            BLK_MS = 0.012
            tc.tile_set_cur_wait(MLP0)
            pa_sb = cp.tile([128, 4, BR], bf16, tag="pa")
            for cc in range(4):
                ps = pm.tile([128, 4, R], f32, tag="pm")
                for th in range(2):
                    for rt in range(2):
                        nc.tensor.matmul(
                            ps[:, 2 * rt:2 * rt + 2, :],
                            w1a[:, th, :, cc * 128:(cc + 1) * 128],
                            predsT[:, 2 * th:2 * th + 2,
                                   rt * 512:(rt + 1) * 512],
                            start=(th == 0), stop=(th == 1), perf_mode=DRM,
                            skip_group_check=True)
                nc.vector.tensor_scalar(pa_sb[:, cc, :], ps[:],
                                        b1[:, cc:cc + 1], None, op0=ALU.add)

            # ------------------------------------------------ blocks
            # 3-stage software pipeline: window w runs L1(w), L2(w-1) and
            # L3(w-2).  The one-window delay between a stage's producers and
            # consumers means no tensor-queue instruction ever waits on an
            # eviction — the PE stays saturated and holds the full clock.
            logits = cp.tile([128, NBLK, 8], f32, tag="logits")
            y1_t, y2_t, ps3_t = {}, {}, {}
            for w in range(NBLK + 2):
                tc.tile_set_cur_wait(MLP0 + (w + 1) * BLK_MS)
                if w < NBLK:
                    b = w
                    if b < NEG:
                        xt = mp.tile([128, 4, BR], f8, tag="negsx", bufs=3)
                        nc.sync.dma_start(out=xt[:], in_=d_negs[b])
                    else:
                        xt = tg
                    y1p = mp.tile([128, 4, BR], bf16, tag="y1p", bufs=2)
                    y1 = mp.tile([128, 4, BR], f8, tag="y1", bufs=3)
                    y1_t[b] = y1
                    for cc in range(4):
                        ps = pm.tile([128, 4, R], f32, tag="pm")
                        l1_matmuls(ps, cc, xt)
                        nc.vector.tensor_add(y1p[:, cc, :], ps[:],
                                             pa_sb[:, cc, :])
                        if cc < 2:
                            nc.scalar.activation(y1[:, cc, :], y1p[:, cc, :],
                                                 AF.Relu)
                        else:
                            nc.vector.tensor_scalar(y1[:, cc, :],
                                                    y1p[:, cc, :],
                                                    0.0, None, op0=ALU.max)
                if 0 <= w - 1 < NBLK:
                    b = w - 1
                    y1 = y1_t.pop(b)
                    y2 = mp.tile([128, 4, BR], f8, tag="y2", bufs=3)
                    y2_t[b] = y2
                    with tc.tile_wait_until(MLP0 + (w + 1) * BLK_MS + 0.004):
                        for cc in range(4):
                            ps = pm.tile([128, 4, R], f32, tag="pm")
                            for th in range(2):
                                for rt in range(2):
                                    nc.tensor.matmul(
                                        ps[:, 2 * rt:2 * rt + 2, :],
                                        w2[:, th, :, cc * 128:(cc + 1) * 128],
                                        y1[:, 2 * th:2 * th + 2,
                                           rt * 512:(rt + 1) * 512],
                                        start=(th == 0), stop=(th == 1),
                                        perf_mode=DRM, skip_group_check=True)
                            nc.scalar.activation(y2[:, cc, :], ps[:], AF.Relu,
                                                 bias=b2[:, cc:cc + 1])
                if 0 <= w - 2 < NBLK:
                    b = w - 2
                    y2 = y2_t.pop(b)
                    ps3 = pa_pool.tile([128, 8], f32, tag="pa3")
                    with tc.tile_wait_until(MLP0 + (w + 1) * BLK_MS + 0.008):
                        for col in range(8):
                            for th in range(2):
                                nc.tensor.matmul(
                                    ps3[:, col:col + 1],
                                    y2[:, 2 * th:2 * th + 2,
                                       col * 128:(col + 1) * 128],
                                    w38[:, th, :, :],
                                    start=(th == 0), stop=(th == 1),
                                    perf_mode=DRM, skip_group_check=True)
                        nc.scalar.activation(logits[:, b, :], ps3[:], AF.Copy)

            # ------------------------------------- softplus + masked sums
            # softplus(t) = relu(t) - ln(sigmoid(|t|)); |t| is sign-agnostic
            # so Abs/Sigmoid/Ln run over all 21 blocks at once.
            tc.tile_set_cur_wait(MLP0 + (NBLK + 4) * BLK_MS)
            sp_a = cp.tile([128, NBLK, 8], f32, tag="sp_a")
            sp_r = cp.tile([128, NBLK, 8], f32, tag="sp_r")
            sp = cp.tile([128, NBLK, 8], f32, tag="sp")
            spn_sum = cp.tile([128, 8], f32, tag="spn_sum")
            junk = cp.tile([128, 3, 8], f32, tag="junk")
            vcol = cp.tile([128, 4], f32, tag="vcol")
            nc.scalar.activation(sp_a[:], logits[:], AF.Abs, bias=b3f)
            nc.scalar.activation(sp_r[:, :NEG, :], logits[:, :NEG, :],
                                 AF.Relu, bias=b3f)
            nc.scalar.activation(sp_r[:, NEG, :], logits[:, NEG, :],
                                 AF.Relu, bias=-b3f, scale=-1.0)
            nc.scalar.activation(sp_a[:], sp_a[:], AF.Sigmoid)
            nc.scalar.activation(sp_a[:], sp_a[:], AF.Ln)
            nc.vector.tensor_sub(sp[:], sp_r[:], sp_a[:])
            nc.vector.tensor_reduce(
                spn_sum[:], sp[:, :NEG, :].transpose([0, 2, 1]),
                mybir.AxisListType.X, ALU.add)
            nc.any.memset(vcol[:], 0.0)
            nc.vector.scalar_tensor_tensor(
                junk[:, 0, :], in0=sp[:, NEG, :], scalar=1.0, in1=mfT[:],
                op0=ALU.mult, op1=ALU.mult, accum_out=vcol[:, 0:1])
            nc.vector.scalar_tensor_tensor(
                junk[:, 1, :], in0=spn_sum[:], scalar=1.0, in1=mfT[:],
                op0=ALU.mult, op1=ALU.mult, accum_out=vcol[:, 1:2])
            nc.vector.tensor_scalar(
                junk[:, 2, :], mfT[:], 1.0, 0.0, op0=ALU.mult,
                op1=ALU.add, accum_out=vcol[:, 2:3])

            ones = cp.tile([128, 1], f32, tag="ones")
            nc.any.memset(ones[:], 1.0)
            psf = pa_pool.tile([128, 8], f32, tag="pa3")
            nc.tensor.matmul(psf[0:1, 0:4], ones[:], vcol[:],
                             start=True, stop=True)
            out_sb = cp.tile([1, 4], f32, tag="out_sb")
            nc.scalar.activation(out_sb[:], psf[0:1, 0:4], AF.Copy)
            nc.sync.dma_start(out=d_out[:], in_=out_sb[:])

    nc.finalize()
    return nc


def _get_program(u_list, k_eff, b3f):
    key = (tuple(u_list), k_eff, float(b3f))
    if key not in _PROGRAM_CACHE:
        _PROGRAM_CACHE[key] = _build_program(u_list, k_eff, b3f)
    return _PROGRAM_CACHE[key]


# ------------------------------------------------------------------ kernel

def kernel(**inputs):
    u_list = [int(x) for x in np.asarray(inputs["unroll_subsample"]).reshape(-1)]
    k_eff = max(u_list) + 1
    shared = _prep_shared(inputs, u_list, k_eff)
    nc = _get_program(u_list, k_eff, shared["b3f"])

    wmaps = {k: v for k, v in shared.items()
             if k in ("w_hh8", "w1a8", "w1b8", "w28", "w38", "bhhg",
                      "b1T", "b2T")}
    in_maps = []
    for c in range(NC):
        m = dict(wmaps)
        m.update(_prep_core(c, inputs, shared, u_list, k_eff))
        in_maps.append(m)

    res = bass_utils.run_bass_kernel_spmd(nc, in_maps, list(range(NC)))
    P = Ng = D = 0.0
    for c in range(NC):
        o = np.asarray(res.results[c]["out"], np.float64)
        P += o[0, 0]
        Ng += o[0, 1]
        D += o[0, 2]
    loss = COEFF * (P / D + Ng / (D * NEG))
    return np.float32(loss)


# revision 26
# speedup vs baseline: 3.8081x; 1.0694x over previous
"""Trainium2 Bass kernel for the CPCA auxiliary loss (nn_CPCA_51754355917033).

Strategy (data-parallel over the env/batch dim n, 16 envs per core):
  - Host side: every gather is baked into per-core contiguous device
    inputs (action-embedding -> gi with b_ih and the r/z part of b_hh
    folded in, h0, targets, negatives, forward mask).
  - GRU: r/z gate adds are plain TT adds (bias pre-folded); the g-gate
    bias is injected with a K=1 ones-matmul so the r* product reads raw
    PSUM; all post-PSUM element-wise work runs in bf16 SBUF (fast DVE
    modes).  The serial per-step tail would leave the PE idle ~4us per
    step (and drop it to the 1.2 GHz p-state): those gaps are filled
    with the L1 x@W1b matmuls of one MLP block per step ("hoisted"
    blocks), keeping the PE at full clock.
  - MLP: preds @ W1a computed once (pa); per block only x @ W1b (16
    MMs, absent for hoisted blocks), L2 (16 MMs), fp8-DoubleRow L3 with
    y2 stationary.  Evictions balanced across DVE and ACT.
  - Tail: batched softplus (relu - ln(sigmoid(|.|))) + accum_out fused
    masked sums.  Host combines the 8 cores' partials.
"""

import numpy as np
import ml_dtypes

import concourse.bass as bass
import concourse.mybir as mybir
import concourse.tile as tile
from concourse import bacc
from concourse import bass_utils

BF16 = ml_dtypes.bfloat16
F8 = ml_dtypes.float8_e4m3
DT = mybir.dt
AF = mybir.ActivationFunctionType
ALU = mybir.AluOpType
DRM = mybir.MatmulPerfMode.DoubleRow

N, T, H, K, S, F, EMB, NLOG, NEG = 128, 512, 512, 16, 16, 4, 32, 18, 20
COEFF = 0.1
NC = 8
NPC = N // NC          # envs per core
R = NPC * S            # GRU rows per core (256)
L = T - 1
NBLK = NEG + 1         # 20 negative g-blocks + 1 positive block
BR = F * R             # rows per block (1024)

_PROGRAM_CACHE = {}


# ----------------------------------------------------------------- host prep

def _prep_shared(inputs, u_list, k_eff):
    """Per-run (not per-core) preprocessing."""
    W_ih = np.asarray(inputs["W_ih"], np.float32)
    W_hh = np.asarray(inputs["W_hh"], np.float32)
    b_ih = np.asarray(inputs["b_ih"], np.float32)
    b_hh = np.asarray(inputs["b_hh"], np.float32)
    W1 = np.asarray(inputs["W1"], np.float32)
    b1 = np.asarray(inputs["b1"], np.float32)
    W2 = np.asarray(inputs["W2"], np.float32)
    b2 = np.asarray(inputs["b2"], np.float32)
    W3 = np.asarray(inputs["W3"], np.float32)
    b3 = np.asarray(inputs["b3"], np.float32)
    emb_tab = np.asarray(inputs["action_embed"], np.float32)

    d = {}
    # GIE: action -> 1536-dim gi with b_ih everywhere and the r/z part of
    # b_hh folded in (its g part sits inside the r* product).
    GIE = np.zeros((NLOG + 1, 1536), np.float32)
    GIE[:NLOG] = emb_tab @ W_ih.T
    GIE += b_ih
    GIE[:, :1024] += b_hh[:1024]
    d["GIE"] = GIE

    d["w_hh8"] = np.ascontiguousarray(
        W_hh.T.reshape(2, 2, 128, 1536).transpose(0, 2, 1, 3)).astype(F8)

    def pack8(WT):
        # [t, ki, ko, m] with contract index = t*256 + ko*128 + ki
        return np.ascontiguousarray(
            WT.reshape(2, 2, 128, WT.shape[1]).transpose(0, 2, 1, 3)).astype(F8)
    d["w1a8"] = pack8(W1[:, :512].T.copy())
    d["w1b8"] = pack8(W1[:, 512:].T.copy())
    d["w28"] = pack8(W2.T.copy())
    # w3 stationary for fp8-DR L3: [128, th, dr, 1], k = th*256 + dr*128 + p
    d["w38"] = np.ascontiguousarray(
        W3[0].reshape(2, 2, 128).transpose(2, 0, 1).reshape(128, 2, 2, 1)
    ).astype(F8)
    # b_hh g part for the K=1 bias matmul: [1, 4, 128] bf16
    d["bhhg"] = np.ascontiguousarray(
        b_hh[1024:].reshape(1, 4, 128)).astype(BF16)
    d["b1T"] = np.ascontiguousarray(b1.reshape(4, 128).T).astype(np.float32)
    d["b2T"] = np.ascontiguousarray(b2.reshape(4, 128).T).astype(np.float32)
    d["b3f"] = float(b3.reshape(-1)[0])

    ti = np.asarray(inputs["time_subsample"]).astype(np.int64)
    idx = np.arange(k_eff)[:, None] + ti[None, :]          # (k_eff, S)
    d["ti"] = ti
    d["idx"] = idx
    return d


def _prep_core(c, inputs, shared, u_list, k_eff):
    acts = np.asarray(inputs["actions"])[..., 0]
    nd = np.asarray(inputs["not_dones"])[..., 0]
    ri = np.asarray(inputs["rnn_inputs"], np.float32)
    ro = np.asarray(inputs["rnn_outputs"], np.float32)
    neg_idx = np.asarray(inputs["neg_idx"]).astype(np.int64)
    ti, idx = shared["ti"], shared["idx"]

    ns = slice(c * NPC, (c + 1) * NPC)

    # gi for all 12 gate chunks: [k_eff, 128, 12, R] bf16
    act_ext = np.full((NPC, L + K), NLOG, np.int64)
    act_ext[:, :L] = acts[ns, :L]
    AI = act_ext[:, idx].transpose(1, 0, 2).reshape(k_eff, R)   # (k_eff, R)
    gi_all = shared["GIE"][AI]                                  # (k_eff, R, 1536)
    giT = np.ascontiguousarray(
        gi_all.transpose(0, 2, 1).reshape(k_eff, 12, 128, R)
        .transpose(0, 2, 1, 3)).astype(BF16)                    # (k_eff,128,12,R)

    H0 = ro[ns][:, ti]                                          # (NPC, S, H)
    h0T = np.ascontiguousarray(
        H0.transpose(2, 0, 1).reshape(4, 128, R)).astype(BF16)

    ri_ext = np.zeros((NPC, L + K, H), np.float32)
    ri_ext[:, :L] = ri[ns, 1:]
    idx2 = np.asarray(u_list)[:, None] + ti[None, :]            # (F, S)
    TG = ri_ext[:, idx2]                                        # (NPC, F, S, H)
    tgT = np.ascontiguousarray(
        TG.transpose(3, 1, 0, 2).reshape(4, 128, BR)).astype(F8)

    ni = neg_idx.reshape(F, N, S, NEG)[:, ns]                   # (F, NPC, S, NEG)
    P = ni.transpose(3, 0, 1, 2).reshape(-1)                    # (g, f, n, s)
    negs = ri.reshape(N * T, H)[P]                              # (NEG*BR, H)
    negsT = np.ascontiguousarray(
        negs.T.reshape(4, 128, NEG, BR).transpose(2, 1, 0, 3)).astype(F8)

    # forward mask on host: mfT [128, 2F] f32, j = 2*fi + half,
    # row = half*128 + p  (rows are (n, s) flattened, 256 per core)
    nd_ext = np.zeros((NPC, L + K), np.float32)
    nd_ext[:, :L] = nd[ns, :L]
    G = (nd_ext[:, idx] > 0)                                    # (NPC, k_eff, S)
    fm = np.cumprod(G.transpose(1, 0, 2).reshape(k_eff, R), axis=0) > 0
    mf = fm[np.asarray(u_list)].astype(np.float32)              # (F, R)
    mfT = np.ascontiguousarray(
        mf.reshape(F, 2, 128).transpose(2, 0, 1).reshape(128, 2 * F)
    ).astype(np.float32)

    return dict(giT=giT, h0T=h0T, tgT=tgT, negsT=negsT, mfT=mfT)


# ------------------------------------------------------------- device program

def _build_program(u_list, k_eff, b3f):
    nc = bacc.Bacc("TRN2", target_bir_lowering=False, debug=False,
                   num_devices=NC)

    f32, bf16, f8 = DT.float32, DT.bfloat16, DT.float8e4

    def inp(name, shape, dt):
        return nc.dram_tensor(name, list(shape), dt, kind="ExternalInput")

    d_whh = inp("w_hh8", (2, 128, 2, 1536), f8)
    d_gi = inp("giT", (k_eff, 128, 12, R), bf16)
    d_h0 = inp("h0T", (4, 128, R), bf16)
    d_w1a = inp("w1a8", (2, 128, 2, 512), f8)
    d_w1b = inp("w1b8", (2, 128, 2, 512), f8)
    d_w2 = inp("w28", (2, 128, 2, 512), f8)
    d_w3 = inp("w38", (128, 2, 2, 1), f8)
    d_bhhg = inp("bhhg", (1, 4, 128), bf16)
    d_b1 = inp("b1T", (128, 4), f32)
    d_b2 = inp("b2T", (128, 4), f32)
    d_tg = inp("tgT", (4, 128, BR), f8)
    d_negs = inp("negsT", (NEG, 128, 4, BR), f8)
    d_mf = inp("mfT", (128, 2 * F), f32)
    d_out = nc.dram_tensor("out", [1, 4], f32, kind="ExternalOutput")

    with tile.TileContext(nc) as tc:
        with (
            tc.tile_pool(name="const", bufs=1) as cp,
            tc.tile_pool(name="gru", bufs=2) as gp,
            tc.tile_pool(name="mlp", bufs=2) as mp,
            tc.tile_pool(name="ps", bufs=3, space="PSUM") as pm,
            tc.tile_pool(name="psa", bufs=2, space="PSUM") as pa_pool,
        ):
            # ---------------------------------------------- constant loads
            # startup-critical first: whh + h0 + bias + gi0 + w1b
            whh = cp.tile([128, 2, 2, 1536], f8, tag="whh")
            for th in range(2):
                nc.sync.dma_start(out=whh[:, th, :, :], in_=d_whh[th])
            h_prev = gp.tile([128, 4, R], bf16, tag="h")
            for kc in range(4):
                nc.sync.dma_start(out=h_prev[:, kc, :], in_=d_h0[kc])
            bhhg = cp.tile([1, 4, 128], bf16, tag="bhhg")
            nc.sync.dma_start(out=bhhg[:], in_=d_bhhg[:])
            ones1 = cp.tile([1, R], bf16, tag="ones1")
            nc.any.memset(ones1[:], 1.0)
            gi_tiles = []
            for k in range(k_eff):
                gt = gp.tile([128, 12, R], bf16, tag="gi", bufs=3)
                gi_tiles.append(gt)
                if k == 0:
                    nc.sync.dma_start(out=gt[:], in_=d_gi[0])
            w1b = cp.tile([128, 2, 2, 512], f8, tag="w1b")
            for th in range(2):
                nc.sync.dma_start(out=w1b[:, th, :, :], in_=d_w1b[th])
            tg = cp.tile([128, 4, BR], f8, tag="tg")
            for kc in range(4):
                nc.sync.dma_start(out=tg[:, kc, :], in_=d_tg[kc])

            # rest of the constants (needed only for pa / MLP phase)
            w1a = cp.tile([128, 2, 2, 512], f8, tag="w1a")
            w2 = cp.tile([128, 2, 2, 512], f8, tag="w2")
            for (t, dd) in ((w1a, d_w1a), (w2, d_w2)):
                for th in range(2):
                    nc.sync.dma_start(out=t[:, th, :, :], in_=dd[th])
            w38 = cp.tile([128, 2, 2, 1], f8, tag="w38")
            nc.sync.dma_start(out=w38[:], in_=d_w3[:])
            b1 = cp.tile([128, 4], f32, tag="b1")
            nc.sync.dma_start(out=b1[:], in_=d_b1[:])
            b2 = cp.tile([128, 4], f32, tag="b2")
            nc.sync.dma_start(out=b2[:], in_=d_b2[:])
            mfT = cp.tile([128, 2 * F], f32, tag="mfT")
            nc.sync.dma_start(out=mfT[:], in_=d_mf[:])

            # ------------------------------------------------ GRU
            h8_prev = gp.tile([128, 4, R], f8, tag="h8")
            nc.vector.tensor_copy(h8_prev[:], h_prev[:])
            predsT = cp.tile([128, 4, BR], f8, tag="preds")

            def l1_matmuls(ps, cc, xt):
                for th in range(2):
                    for rt in range(2):
                        nc.tensor.matmul(
                            ps[:, 2 * rt:2 * rt + 2, :],
                            w1b[:, th, :, cc * 128:(cc + 1) * 128],
                            xt[:, 2 * th:2 * th + 2,
                               rt * 512:(rt + 1) * 512],
                            start=(th == 0), stop=(th == 1),
                            perf_mode=DRM, skip_group_check=True)

            # manual schedule ladder: the greedy scheduler (whose sim
            # under-models the hardware) otherwise reorders within engine
            # queues in ways that stretch the serial GRU tail.
            STEP_MS = 0.02
            DUMMIES = 16

            for k in range(k_eff):
                gi = gi_tiles[k]
                tc.tile_set_cur_wait(k * STEP_MS + 0.001)
                if k + 1 < k_eff:
                    nc.sync.dma_start(out=gi_tiles[k + 1][:], in_=d_gi[k + 1])

                psr = pm.tile([128, 4, R], f32, tag="pm")
                psz = pm.tile([128, 4, R], f32, tag="pm")
                psg = pm.tile([128, 4, R], f32, tag="pm")
                rp_sb = gp.tile([128, 4, R], bf16, tag="rp", bufs=1)
                zp_sb = gp.tile([128, 4, R], bf16, tag="zp", bufs=1)
                r_sb = gp.tile([128, 4, R], bf16, tag="r", bufs=1)
                z_sb = gp.tile([128, 4, R], bf16, tag="z", bufs=1)
                t_sb = gp.tile([128, 4, R], bf16, tag="t", bufs=1)
                u_sb = gp.tile([128, 4, R], bf16, tag="u", bufs=1)
                g_sb = gp.tile([128, 4, R], bf16, tag="g", bufs=1)
                e_sb = gp.tile([128, 4, R], bf16, tag="e", bufs=1)
                w1m = gp.tile([128, 4, R], bf16, tag="w1m", bufs=1)
                gw = gp.tile([128, 4, R], bf16, tag="gw", bufs=1)
                h8_new = gp.tile([128, 4, R], f8, tag="h8")

                # tensor queue: r(8), z(8), bias-g(4)+g(8), then dummy fill
                for pst, base in ((psr, 0), (psz, 4)):
                    for c in range(4):
                        gc = base + c
                        for th in range(2):
                            nc.tensor.matmul(
                                pst[:, c, :],
                                whh[:, th, :, gc * 128:(gc + 1) * 128],
                                h8_prev[:, 2 * th:2 * th + 2, :],
                                start=(th == 0), stop=(th == 1),
                                perf_mode=DRM, skip_group_check=True)
                for c in range(4):
                    nc.tensor.matmul(
                        psg[:, c, :], bhhg[:, c, :], ones1[:],
                        start=True, stop=False, skip_group_check=True)
                    gc = 8 + c
                    for th in range(2):
                        nc.tensor.matmul(
                            psg[:, c, :],
                            whh[:, th, :, gc * 128:(gc + 1) * 128],
                            h8_prev[:, 2 * th:2 * th + 2, :],
                            start=False, stop=(th == 1), perf_mode=DRM,
                            skip_group_check=True)

                # DVE: rz adds in bf16, then the serial tail (h kept in
                # f8; t/u/tanh split in kc halves to overlap ACT and DVE)
                nc.vector.tensor_add(rp_sb[:], psr[:], gi[:, 0:4, :])
                nc.vector.tensor_add(zp_sb[:], psz[:], gi[:, 4:8, :])
                nc.scalar.activation(r_sb[:], rp_sb[:], AF.Sigmoid)
                nc.scalar.activation(z_sb[:], zp_sb[:], AF.Sigmoid)
                for hh in range(2):
                    s = slice(2 * hh, 2 * hh + 2)
                    nc.vector.tensor_mul(t_sb[:, s, :], psg[:, s, :],
                                         r_sb[:, s, :])
                    nc.vector.tensor_add(u_sb[:, s, :], t_sb[:, s, :],
                                         gi[:, 8 + 2 * hh:10 + 2 * hh, :])
                    nc.scalar.activation(g_sb[:, s, :], u_sb[:, s, :],
                                         AF.Tanh)
                nc.gpsimd.tensor_scalar(w1m[:], z_sb[:], -1.0, 1.0,
                                        op0=ALU.mult, op1=ALU.add)
                nc.vector.tensor_mul(e_sb[:], z_sb[:], h8_prev[:])
                nc.vector.tensor_mul(gw[:], g_sb[:], w1m[:])
                nc.vector.tensor_add(h8_new[:], gw[:], e_sb[:])

                # dummy matmuls (outputs never read): keep the PE busy
                # through the serial tail so the p-state governor holds the
                # full clock; at half clock every real matmul costs ~1.6x.
                with tc.tile_wait_until(k * STEP_MS + 0.010):
                    psd = pm.tile([128, 4, R], f32, tag="pm")
                    for dmy in range(DUMMIES):
                        nc.tensor.matmul(
                            psd[:, 2 * (dmy % 2):2 * (dmy % 2) + 2, :],
                            w1b[:, 0, :, 0:128],
                            tg[:, 0:2, 0:512],
                            start=True, stop=True, perf_mode=DRM,
                            skip_group_check=True)

                h8_prev = h8_new
                for fi, u in enumerate(u_list):
                    if u == k:
                        with tc.tile_wait_until(k * STEP_MS + 0.016):
                            nc.scalar.activation(
                                predsT[:, :, fi * R:(fi + 1) * R],
                                h8_new[:], AF.Copy)

            # ------------------------------------------------ pa = preds@W1a
            MLP0 = k_eff * STEP_MS + 0.005
            BLK_MS = 0.012
            tc.tile_set_cur_wait(MLP0)
            pa_sb = cp.tile([128, 4, BR], bf16, tag="pa")
            for cc in range(4):
                ps = pm.tile([128, 4, R], f32, tag="pm")
                for th in range(2):
                    for rt in range(2):
                        nc.tensor.matmul(
                            ps[:, 2 * rt:2 * rt + 2, :],
                            w1a[:, th, :, cc * 128:(cc + 1) * 128],
                            predsT[:, 2 * th:2 * th + 2,
                                   rt * 512:(rt + 1) * 512],
                            start=(th == 0), stop=(th == 1), perf_mode=DRM,
                            skip_group_check=True)
                nc.vector.tensor_scalar(pa_sb[:, cc, :], ps[:],
                                        b1[:, cc:cc + 1], None, op0=ALU.add)

            # ------------------------------------------------ blocks
            # 3-stage software pipeline: window w runs L1(w), L2(w-1) and
            # L3(w-2).  The one-window delay between a stage's producers and
            # consumers means no tensor-queue instruction ever waits on an
            # eviction — the PE stays saturated and holds the full clock.
            logits = cp.tile([128, NBLK, 8], f32, tag="logits")
            y1_t, y2_t, ps3_t = {}, {}, {}
            for w in range(NBLK + 2):
                tc.tile_set_cur_wait(MLP0 + (w + 1) * BLK_MS)
                if w < NBLK:
                    b = w
                    if b < NEG:
                        xt = mp.tile([128, 4, BR], f8, tag="negsx", bufs=3)
                        nc.sync.dma_start(out=xt[:], in_=d_negs[b])
                    else:
                        xt = tg
                    y1p = mp.tile([128, 4, BR], bf16, tag="y1p", bufs=2)
                    y1 = mp.tile([128, 4, BR], f8, tag="y1", bufs=3)
                    y1_t[b] = y1
                    for cc in range(4):
                        ps = pm.tile([128, 4, R], f32, tag="pm")
                        l1_matmuls(ps, cc, xt)
                        nc.vector.tensor_add(y1p[:, cc, :], ps[:],
                                             pa_sb[:, cc, :])
                        if cc < 2:
                            nc.scalar.activation(y1[:, cc, :], y1p[:, cc, :],
                                                 AF.Relu)
                        else:
                            nc.vector.tensor_scalar(y1[:, cc, :],
                                                    y1p[:, cc, :],
                                                    0.0, None, op0=ALU.max)
                if 0 <= w - 1 < NBLK:
                    b = w - 1
                    y1 = y1_t.pop(b)
                    y2 = mp.tile([128, 4, BR], f8, tag="y2", bufs=3)
                    y2_t[b] = y2
                    with tc.tile_wait_until(MLP0 + (w + 1) * BLK_MS + 0.004):
                        for cc in range(4):
                            ps = pm.tile([128, 4, R], f32, tag="pm")
                            for th in range(2):
                                for rt in range(2):
                                    nc.tensor.matmul(
                                        ps[:, 2 * rt:2 * rt + 2, :],
                                        w2[:, th, :, cc * 128:(cc + 1) * 128],
                                        y1[:, 2 * th:2 * th + 2,
                                           rt * 512:(rt + 1) * 512],
                                        start=(th == 0), stop=(th == 1),
                                        perf_mode=DRM, skip_group_check=True)
                            nc.scalar.activation(y2[:, cc, :], ps[:], AF.Relu,
                                                 bias=b2[:, cc:cc + 1])
                if 0 <= w - 2 < NBLK:
                    b = w - 2
                    y2 = y2_t.pop(b)
                    ps3 = pa_pool.tile([128, 8], f32, tag="pa3")
                    with tc.tile_wait_until(MLP0 + (w + 1) * BLK_MS + 0.008):
                        for col in range(8):
                            for th in range(2):
                                nc.tensor.matmul(
                                    ps3[:, col:col + 1],
                                    y2[:, 2 * th:2 * th + 2,
                                       col * 128:(col + 1) * 128],
                                    w38[:, th, :, :],
                                    start=(th == 0), stop=(th == 1),
                                    perf_mode=DRM, skip_group_check=True)
                        nc.scalar.activation(logits[:, b, :], ps3[:], AF.Copy)

            # ------------------------------------- softplus + masked sums
            # softplus(t) = relu(t) - ln(sigmoid(|t|)); |t| is sign-agnostic
            # so Abs/Sigmoid/Ln run over all 21 blocks at once.
            tc.tile_set_cur_wait(MLP0 + (NBLK + 4) * BLK_MS)
            sp_a = cp.tile([128, NBLK, 8], f32, tag="sp_a")
            sp_r = cp.tile([128, NBLK, 8], f32, tag="sp_r")
            sp = cp.tile([128, NBLK, 8], f32, tag="sp")
            spn_sum = cp.tile([128, 8], f32, tag="spn_sum")
            junk = cp.tile([128, 3, 8], f32, tag="junk")
            vcol = cp.tile([128, 4], f32, tag="vcol")
            nc.scalar.activation(sp_a[:], logits[:], AF.Abs, bias=b3f)
            nc.scalar.activation(sp_r[:, :NEG, :], logits[:, :NEG, :],
                                 AF.Relu, bias=b3f)
            nc.scalar.activation(sp_r[:, NEG, :], logits[:, NEG, :],
                                 AF.Relu, bias=-b3f, scale=-1.0)
            nc.scalar.activation(sp_a[:], sp_a[:], AF.Sigmoid)
            nc.scalar.activation(sp_a[:], sp_a[:], AF.Ln)
            nc.vector.tensor_sub(sp[:], sp_r[:], sp_a[:])
            nc.vector.tensor_reduce(
                spn_sum[:], sp[:, :NEG, :].transpose([0, 2, 1]),
                mybir.AxisListType.X, ALU.add)
            nc.any.memset(vcol[:], 0.0)
            nc.vector.scalar_tensor_tensor(
                junk[:, 0, :], in0=sp[:, NEG, :], scalar=1.0, in1=mfT[:],
                op0=ALU.mult, op1=ALU.mult, accum_out=vcol[:, 0:1])
            nc.vector.scalar_tensor_tensor(
                junk[:, 1, :], in0=spn_sum[:], scalar=1.0, in1=mfT[:],
                op0=ALU.mult, op1=ALU.mult, accum_out=vcol[:, 1:2])
            nc.vector.tensor_scalar(
                junk[:, 2, :], mfT[:], 1.0, 0.0, op0=ALU.mult,
                op1=ALU.add, accum_out=vcol[:, 2:3])

            ones = cp.tile([128, 1], f32, tag="ones")
            nc.any.memset(ones[:], 1.0)
            psf = pa_pool.tile([128, 8], f32, tag="pa3")
            nc.tensor.matmul(psf[0:1, 0:4], ones[:], vcol[:],
                             start=True, stop=True)
            out_sb = cp.tile([1, 4], f32, tag="out_sb")
            nc.scalar.activation(out_sb[:], psf[0:1, 0:4], AF.Copy)
            nc.sync.dma_start(out=d_out[:], in_=out_sb[:])

    nc.finalize()
    return nc


def _get_program(u_list, k_eff, b3f):
    key = (tuple(u_list), k_eff, float(b3f))
    if key not in _PROGRAM_CACHE:
        _PROGRAM_CACHE[key] = _build_program(u_list, k_eff, b3f)
    return _PROGRAM_CACHE[key]


# ------------------------------------------------------------------ kernel

def kernel(**inputs):
    u_list = [int(x) for x in np.asarray(inputs["unroll_subsample"]).reshape(-1)]
    k_eff = max(u_list) + 1
    shared = _prep_shared(inputs, u_list, k_eff)
    nc = _get_program(u_list, k_eff, shared["b3f"])

    wmaps = {k: v for k, v in shared.items()
             if k in ("w_hh8", "w1a8", "w1b8", "w28", "w38", "bhhg",
                      "b1T", "b2T")}
    in_maps = []
    for c in range(NC):
        m = dict(wmaps)
        m.update(_prep_core(c, inputs, shared, u_list, k_eff))
        in_maps.append(m)

    res = bass_utils.run_bass_kernel_spmd(nc, in_maps, list(range(NC)))
    P = Ng = D = 0.0
    for c in range(NC):
        o = np.asarray(res.results[c]["out"], np.float64)
        P += o[0, 0]
        Ng += o[0, 1]
        D += o[0, 2]
    loss = COEFF * (P / D + Ng / (D * NEG))
    return np.float32(loss)


# revision 29
# speedup vs baseline: 3.9561x; 1.0389x over previous
"""Trainium2 Bass kernel for the CPCA auxiliary loss (nn_CPCA_51754355917033).

Strategy (data-parallel over the env/batch dim n, 16 envs per core):
  - Host side: every gather is baked into per-core contiguous device
    inputs (action-embedding -> gi with b_ih and the r/z part of b_hh
    folded in, h0, targets, negatives, forward mask).
  - GRU: r/z gate adds are plain TT adds (bias pre-folded); the g-gate
    bias is injected with a K=1 ones-matmul so the r* product reads raw
    PSUM; all post-PSUM element-wise work runs in bf16 SBUF (fast DVE
    modes).  The serial per-step tail would leave the PE idle ~4us per
    step (and drop it to the 1.2 GHz p-state): those gaps are filled
    with the L1 x@W1b matmuls of one MLP block per step ("hoisted"
    blocks), keeping the PE at full clock.
  - MLP: preds @ W1a computed once (pa); per block only x @ W1b (16
    MMs, absent for hoisted blocks), L2 (16 MMs), fp8-DoubleRow L3 with
    y2 stationary.  Evictions balanced across DVE and ACT.
  - Tail: batched softplus (relu - ln(sigmoid(|.|))) + accum_out fused
    masked sums.  Host combines the 8 cores' partials.
"""

import numpy as np
import ml_dtypes

import concourse.bass as bass
import concourse.mybir as mybir
import concourse.tile as tile
from concourse import bacc
from concourse import bass_utils

BF16 = ml_dtypes.bfloat16
F8 = ml_dtypes.float8_e4m3
DT = mybir.dt
AF = mybir.ActivationFunctionType
ALU = mybir.AluOpType
DRM = mybir.MatmulPerfMode.DoubleRow

N, T, H, K, S, F, EMB, NLOG, NEG = 128, 512, 512, 16, 16, 4, 32, 18, 20
COEFF = 0.1
NC = 8
NPC = N // NC          # envs per core
R = NPC * S            # GRU rows per core (256)
L = T - 1
NBLK = NEG + 1         # 20 negative g-blocks + 1 positive block
BR = F * R             # rows per block (1024)

_PROGRAM_CACHE = {}


# ----------------------------------------------------------------- host prep

def _prep_shared(inputs, u_list, k_eff):
    """Per-run (not per-core) preprocessing."""
    W_ih = np.asarray(inputs["W_ih"], np.float32)
    W_hh = np.asarray(inputs["W_hh"], np.float32)
    b_ih = np.asarray(inputs["b_ih"], np.float32)
    b_hh = np.asarray(inputs["b_hh"], np.float32)
    W1 = np.asarray(inputs["W1"], np.float32)
    b1 = np.asarray(inputs["b1"], np.float32)
    W2 = np.asarray(inputs["W2"], np.float32)
    b2 = np.asarray(inputs["b2"], np.float32)
    W3 = np.asarray(inputs["W3"], np.float32)
    b3 = np.asarray(inputs["b3"], np.float32)
    emb_tab = np.asarray(inputs["action_embed"], np.float32)

    d = {}
    # GIE: action -> 1536-dim gi with b_ih everywhere and the r/z part of
    # b_hh folded in (its g part sits inside the r* product).
    GIE = np.zeros((NLOG + 1, 1536), np.float32)
    GIE[:NLOG] = emb_tab @ W_ih.T
    GIE += b_ih
    GIE[:, :1024] += b_hh[:1024]
    d["GIE"] = GIE

    d["w_hh8"] = np.ascontiguousarray(
        W_hh.T.reshape(2, 2, 128, 1536).transpose(0, 2, 1, 3)).astype(F8)

    def pack8(WT):
        # [t, ki, ko, m] with contract index = t*256 + ko*128 + ki
        return np.ascontiguousarray(
            WT.reshape(2, 2, 128, WT.shape[1]).transpose(0, 2, 1, 3)).astype(F8)
    d["w1a8"] = pack8(W1[:, :512].T.copy())
    d["w1b8"] = pack8(W1[:, 512:].T.copy())
    d["w28"] = pack8(W2.T.copy())
    # w3 stationary for fp8-DR L3: [128, th, dr, 1], k = th*256 + dr*128 + p
    d["w38"] = np.ascontiguousarray(
        W3[0].reshape(2, 2, 128).transpose(2, 0, 1).reshape(128, 2, 2, 1)
    ).astype(F8)
    # b_hh g part for the K=1 bias matmul: [1, 4, 128] bf16
    d["bhhg"] = np.ascontiguousarray(
        b_hh[1024:].reshape(1, 4, 128)).astype(BF16)
    d["b1T"] = np.ascontiguousarray(b1.reshape(4, 128).T).astype(np.float32)
    d["b2T"] = np.ascontiguousarray(b2.reshape(4, 128).T).astype(np.float32)
    d["b3f"] = float(b3.reshape(-1)[0])

    ti = np.asarray(inputs["time_subsample"]).astype(np.int64)
    idx = np.arange(k_eff)[:, None] + ti[None, :]          # (k_eff, S)
    d["ti"] = ti
    d["idx"] = idx
    return d


def _prep_core(c, inputs, shared, u_list, k_eff):
    acts = np.asarray(inputs["actions"])[..., 0]
    nd = np.asarray(inputs["not_dones"])[..., 0]
    ri = np.asarray(inputs["rnn_inputs"], np.float32)
    ro = np.asarray(inputs["rnn_outputs"], np.float32)
    neg_idx = np.asarray(inputs["neg_idx"]).astype(np.int64)
    ti, idx = shared["ti"], shared["idx"]

    ns = slice(c * NPC, (c + 1) * NPC)

    # gi for all 12 gate chunks: [k_eff, 128, 12, R] bf16
    act_ext = np.full((NPC, L + K), NLOG, np.int64)
    act_ext[:, :L] = acts[ns, :L]
    AI = act_ext[:, idx].transpose(1, 0, 2).reshape(k_eff, R)   # (k_eff, R)
    gi_all = shared["GIE"][AI]                                  # (k_eff, R, 1536)
    giT = np.ascontiguousarray(
        gi_all.transpose(0, 2, 1).reshape(k_eff, 12, 128, R)
        .transpose(0, 2, 1, 3)).astype(BF16)                    # (k_eff,128,12,R)

    H0 = ro[ns][:, ti]                                          # (NPC, S, H)
    h0T = np.ascontiguousarray(
        H0.transpose(2, 0, 1).reshape(4, 128, R)).astype(BF16)

    ri_ext = np.zeros((NPC, L + K, H), np.float32)
    ri_ext[:, :L] = ri[ns, 1:]
    idx2 = np.asarray(u_list)[:, None] + ti[None, :]            # (F, S)
    TG = ri_ext[:, idx2]                                        # (NPC, F, S, H)
    tgT = np.ascontiguousarray(
        TG.transpose(3, 1, 0, 2).reshape(4, 128, BR)).astype(F8)

    ni = neg_idx.reshape(F, N, S, NEG)[:, ns]                   # (F, NPC, S, NEG)
    P = ni.transpose(3, 0, 1, 2).reshape(-1)                    # (g, f, n, s)
    negs = ri.reshape(N * T, H)[P]                              # (NEG*BR, H)
    negsT = np.ascontiguousarray(
        negs.T.reshape(4, 128, NEG, BR).transpose(2, 1, 0, 3)).astype(F8)

    # forward mask on host: mfT [128, 2F] f32, j = 2*fi + half,
    # row = half*128 + p  (rows are (n, s) flattened, 256 per core)
    nd_ext = np.zeros((NPC, L + K), np.float32)
    nd_ext[:, :L] = nd[ns, :L]
    G = (nd_ext[:, idx] > 0)                                    # (NPC, k_eff, S)
    fm = np.cumprod(G.transpose(1, 0, 2).reshape(k_eff, R), axis=0) > 0
    mf = fm[np.asarray(u_list)].astype(np.float32)              # (F, R)
    mfT = np.ascontiguousarray(
        mf.reshape(F, 2, 128).transpose(2, 0, 1).reshape(128, 2 * F)
    ).astype(np.float32)

    return dict(giT=giT, h0T=h0T, tgT=tgT, negsT=negsT, mfT=mfT)


# ------------------------------------------------------------- device program

def _build_program(u_list, k_eff, b3f):
    nc = bacc.Bacc("TRN2", target_bir_lowering=False, debug=False,
                   num_devices=NC)

    f32, bf16, f8 = DT.float32, DT.bfloat16, DT.float8e4

    def inp(name, shape, dt):
        return nc.dram_tensor(name, list(shape), dt, kind="ExternalInput")

    d_whh = inp("w_hh8", (2, 128, 2, 1536), f8)
    d_gi = inp("giT", (k_eff, 128, 12, R), bf16)
    d_h0 = inp("h0T", (4, 128, R), bf16)
    d_w1a = inp("w1a8", (2, 128, 2, 512), f8)
    d_w1b = inp("w1b8", (2, 128, 2, 512), f8)
    d_w2 = inp("w28", (2, 128, 2, 512), f8)
    d_w3 = inp("w38", (128, 2, 2, 1), f8)
    d_bhhg = inp("bhhg", (1, 4, 128), bf16)
    d_b1 = inp("b1T", (128, 4), f32)
    d_b2 = inp("b2T", (128, 4), f32)
    d_tg = inp("tgT", (4, 128, BR), f8)
    d_negs = inp("negsT", (NEG, 128, 4, BR), f8)
    d_mf = inp("mfT", (128, 2 * F), f32)
    d_out = nc.dram_tensor("out", [1, 4], f32, kind="ExternalOutput")

    with tile.TileContext(nc) as tc:
        with (
            tc.tile_pool(name="const", bufs=1) as cp,
            tc.tile_pool(name="gru", bufs=2) as gp,
            tc.tile_pool(name="mlp", bufs=2) as mp,
            tc.tile_pool(name="ps", bufs=3, space="PSUM") as pm,
            tc.tile_pool(name="psa", bufs=2, space="PSUM") as pa_pool,
        ):
            # ---------------------------------------------- constant loads
            # startup-critical first: whh + h0 + bias + gi0 + w1b
            whh = cp.tile([128, 2, 2, 1536], f8, tag="whh")
            for th in range(2):
                nc.sync.dma_start(out=whh[:, th, :, :], in_=d_whh[th])
            h_prev = gp.tile([128, 4, R], bf16, tag="h")
            for kc in range(4):
                nc.sync.dma_start(out=h_prev[:, kc, :], in_=d_h0[kc])
            bhhg = cp.tile([1, 4, 128], bf16, tag="bhhg")
            nc.sync.dma_start(out=bhhg[:], in_=d_bhhg[:])
            ones1 = cp.tile([1, R], bf16, tag="ones1")
            nc.any.memset(ones1[:], 1.0)
            gi_tiles = []
            for k in range(k_eff):
                gt = gp.tile([128, 12, R], bf16, tag="gi", bufs=3)
                gi_tiles.append(gt)
                if k == 0:
                    nc.sync.dma_start(out=gt[:], in_=d_gi[0])
            w1b = cp.tile([128, 2, 2, 512], f8, tag="w1b")
            for th in range(2):
                nc.sync.dma_start(out=w1b[:, th, :, :], in_=d_w1b[th])
            tg = cp.tile([128, 4, BR], f8, tag="tg")
            for kc in range(4):
                nc.sync.dma_start(out=tg[:, kc, :], in_=d_tg[kc])

            # rest of the constants (needed only for pa / MLP phase)
            w1a = cp.tile([128, 2, 2, 512], f8, tag="w1a")
            w2 = cp.tile([128, 2, 2, 512], f8, tag="w2")
            for (t, dd) in ((w1a, d_w1a), (w2, d_w2)):
                for th in range(2):
                    nc.sync.dma_start(out=t[:, th, :, :], in_=dd[th])
            w38 = cp.tile([128, 2, 2, 1], f8, tag="w38")
            nc.sync.dma_start(out=w38[:], in_=d_w3[:])
            b1 = cp.tile([128, 4], f32, tag="b1")
            nc.sync.dma_start(out=b1[:], in_=d_b1[:])
            b2 = cp.tile([128, 4], f32, tag="b2")
            nc.sync.dma_start(out=b2[:], in_=d_b2[:])
            mfT = cp.tile([128, 2 * F], f32, tag="mfT")
            nc.sync.dma_start(out=mfT[:], in_=d_mf[:])

            # ------------------------------------------------ GRU
            h8_prev = gp.tile([128, 4, R], f8, tag="h8")
            nc.vector.tensor_copy(h8_prev[:], h_prev[:])
            predsT = cp.tile([128, 4, BR], f8, tag="preds")

            def l1_matmuls(ps, cc, xt):
                for th in range(2):
                    for rt in range(2):
                        nc.tensor.matmul(
                            ps[:, 2 * rt:2 * rt + 2, :],
                            w1b[:, th, :, cc * 128:(cc + 1) * 128],
                            xt[:, 2 * th:2 * th + 2,
                               rt * 512:(rt + 1) * 512],
                            start=(th == 0), stop=(th == 1),
                            perf_mode=DRM, skip_group_check=True)

            # manual schedule ladder: the greedy scheduler (whose sim
            # under-models the hardware) otherwise reorders within engine
            # queues in ways that stretch the serial GRU tail.
            STEP_MS = 0.02
            DUMMIES = 22

            for k in range(k_eff):
                gi = gi_tiles[k]
                tc.tile_set_cur_wait(k * STEP_MS + 0.001)
                if k + 1 < k_eff:
                    nc.sync.dma_start(out=gi_tiles[k + 1][:], in_=d_gi[k + 1])

                psr = pm.tile([128, 4, R], f32, tag="pm")
                psz = pm.tile([128, 4, R], f32, tag="pm")
                psg = pm.tile([128, 4, R], f32, tag="pm")
                rp_sb = gp.tile([128, 4, R], bf16, tag="rp", bufs=1)
                zp_sb = gp.tile([128, 4, R], bf16, tag="zp", bufs=1)
                r_sb = gp.tile([128, 4, R], bf16, tag="r", bufs=1)
                z_sb = gp.tile([128, 4, R], bf16, tag="z", bufs=1)
                t_sb = gp.tile([128, 4, R], bf16, tag="t", bufs=1)
                u_sb = gp.tile([128, 4, R], bf16, tag="u", bufs=1)
                g_sb = gp.tile([128, 4, R], bf16, tag="g", bufs=1)
                e_sb = gp.tile([128, 4, R], bf16, tag="e", bufs=1)
                w1m = gp.tile([128, 4, R], bf16, tag="w1m", bufs=1)
                gw = gp.tile([128, 4, R], bf16, tag="gw", bufs=1)
                h8_new = gp.tile([128, 4, R], f8, tag="h8")

                # tensor queue: r(8), z(8), bias-g(4)+g(8), then dummy fill.
                # th0 emitted for all chunks before th1 so these matmuls can
                # start as soon as the previous step's h8 low half lands.
                for pst, base in ((psr, 0), (psz, 4)):
                    for th in range(2):
                        for c in range(4):
                            gc = base + c
                            nc.tensor.matmul(
                                pst[:, c, :],
                                whh[:, th, :, gc * 128:(gc + 1) * 128],
                                h8_prev[:, 2 * th:2 * th + 2, :],
                                start=(th == 0), stop=(th == 1),
                                perf_mode=DRM, skip_group_check=True)
                for c in range(4):
                    nc.tensor.matmul(
                        psg[:, c, :], bhhg[:, c, :], ones1[:],
                        start=True, stop=False, skip_group_check=True)
                    gc = 8 + c
                    for th in range(2):
                        nc.tensor.matmul(
                            psg[:, c, :],
                            whh[:, th, :, gc * 128:(gc + 1) * 128],
                            h8_prev[:, 2 * th:2 * th + 2, :],
                            start=False, stop=(th == 1), perf_mode=DRM,
                            skip_group_check=True)

                # DVE: rz adds in bf16, then the serial tail (h kept in
                # f8; t/u/tanh split in kc halves to overlap ACT and DVE)
                nc.vector.tensor_add(rp_sb[:], psr[:], gi[:, 0:4, :])
                nc.vector.tensor_add(zp_sb[:], psz[:], gi[:, 4:8, :])
                nc.scalar.activation(r_sb[:], rp_sb[:], AF.Sigmoid)
                nc.scalar.activation(z_sb[:], zp_sb[:], AF.Sigmoid)
                for hh in range(2):
                    s = slice(2 * hh, 2 * hh + 2)
                    nc.vector.tensor_mul(t_sb[:, s, :], psg[:, s, :],
                                         r_sb[:, s, :])
                    nc.vector.tensor_add(u_sb[:, s, :], t_sb[:, s, :],
                                         gi[:, 8 + 2 * hh:10 + 2 * hh, :])
                    nc.scalar.activation(g_sb[:, s, :], u_sb[:, s, :],
                                         AF.Tanh)
                nc.gpsimd.tensor_scalar(w1m[:], z_sb[:], -1.0, 1.0,
                                        op0=ALU.mult, op1=ALU.add)
                nc.vector.tensor_mul(e_sb[:], z_sb[:], h8_prev[:])
                nc.vector.tensor_mul(gw[:], g_sb[:], w1m[:])
                # h8 written in kc halves so the next step's th0 matmuls
                # (which read only kc 0-1) start while the top half finishes
                nc.vector.tensor_add(h8_new[:, 0:2, :], gw[:, 0:2, :],
                                     e_sb[:, 0:2, :])
                nc.vector.tensor_add(h8_new[:, 2:4, :], gw[:, 2:4, :],
                                     e_sb[:, 2:4, :])

                # dummy matmuls (outputs never read): keep the PE busy
                # through the serial tail so the p-state governor holds a
                # high clock; at low clock every real matmul costs ~2x.
                with tc.tile_wait_until(k * STEP_MS + 0.008):
                    psd = pm.tile([128, 4, R], f32, tag="pm")
                    for dmy in range(DUMMIES):
                        nc.tensor.matmul(
                            psd[:, 2 * (dmy % 2):2 * (dmy % 2) + 2, :],
                            w1b[:, 0, :, 0:128],
                            tg[:, 0:2, 0:512],
                            start=True, stop=True, perf_mode=DRM,
                            skip_group_check=True)

                h8_prev = h8_new
                for fi, u in enumerate(u_list):
                    if u == k:
                        with tc.tile_wait_until(k * STEP_MS + 0.016):
                            nc.scalar.activation(
                                predsT[:, :, fi * R:(fi + 1) * R],
                                h8_new[:], AF.Copy)

            # ------------------------------------------------ pa = preds@W1a
            MLP0 = k_eff * STEP_MS + 0.005
            BLK_MS = 0.012
            tc.tile_set_cur_wait(MLP0)
            pa_sb = cp.tile([128, 4, BR], bf16, tag="pa")
            for cc in range(4):
                ps = pm.tile([128, 4, R], f32, tag="pm")
                for th in range(2):
                    for rt in range(2):
                        nc.tensor.matmul(
                            ps[:, 2 * rt:2 * rt + 2, :],
                            w1a[:, th, :, cc * 128:(cc + 1) * 128],
                            predsT[:, 2 * th:2 * th + 2,
                                   rt * 512:(rt + 1) * 512],
                            start=(th == 0), stop=(th == 1), perf_mode=DRM,
                            skip_group_check=True)
                nc.vector.tensor_scalar(pa_sb[:, cc, :], ps[:],
                                        b1[:, cc:cc + 1], None, op0=ALU.add)

            # ------------------------------------------------ blocks
            # 3-stage software pipeline: window w runs L1(w), L2(w-1) and
            # L3(w-2).  The one-window delay between a stage's producers and
            # consumers means no tensor-queue instruction ever waits on an
            # eviction — the PE stays saturated and holds the full clock.
            logits = cp.tile([128, NBLK, 8], f32, tag="logits")
            y1_t, y2_t, ps3_t = {}, {}, {}
            for w in range(NBLK + 2):
                tc.tile_set_cur_wait(MLP0 + (w + 1) * BLK_MS)
                if w < NBLK:
                    b = w
                    if b < NEG:
                        xt = mp.tile([128, 4, BR], f8, tag="negsx", bufs=3)
                        nc.sync.dma_start(out=xt[:], in_=d_negs[b])
                    else:
                        xt = tg
                    y1p = mp.tile([128, 4, BR], bf16, tag="y1p", bufs=2)
                    y1 = mp.tile([128, 4, BR], f8, tag="y1", bufs=3)
                    y1_t[b] = y1
                    for cc in range(4):
                        ps = pm.tile([128, 4, R], f32, tag="pm")
                        l1_matmuls(ps, cc, xt)
                        nc.vector.tensor_add(y1p[:, cc, :], ps[:],
                                             pa_sb[:, cc, :])
                        if cc < 2:
                            nc.scalar.activation(y1[:, cc, :], y1p[:, cc, :],
                                                 AF.Relu)
                        else:
                            nc.vector.tensor_scalar(y1[:, cc, :],
                                                    y1p[:, cc, :],
                                                    0.0, None, op0=ALU.max)
                if 0 <= w - 1 < NBLK:
                    b = w - 1
                    y1 = y1_t.pop(b)
                    y2 = mp.tile([128, 4, BR], f8, tag="y2", bufs=3)
                    y2_t[b] = y2
                    with tc.tile_wait_until(MLP0 + (w + 1) * BLK_MS + 0.004):
                        for cc in range(4):
                            ps = pm.tile([128, 4, R], f32, tag="pm")
                            for th in range(2):
                                for rt in range(2):
                                    nc.tensor.matmul(
                                        ps[:, 2 * rt:2 * rt + 2, :],
                                        w2[:, th, :, cc * 128:(cc + 1) * 128],
                                        y1[:, 2 * th:2 * th + 2,
                                           rt * 512:(rt + 1) * 512],
                                        start=(th == 0), stop=(th == 1),
                                        perf_mode=DRM, skip_group_check=True)
                            nc.scalar.activation(y2[:, cc, :], ps[:], AF.Relu,
                                                 bias=b2[:, cc:cc + 1])
                if 0 <= w - 2 < NBLK:
                    b = w - 2
                    y2 = y2_t.pop(b)
                    ps3 = pa_pool.tile([128, 8], f32, tag="pa3")
                    with tc.tile_wait_until(MLP0 + (w + 1) * BLK_MS + 0.008):
                        for col in range(8):
                            for th in range(2):
                                nc.tensor.matmul(
                                    ps3[:, col:col + 1],
                                    y2[:, 2 * th:2 * th + 2,
                                       col * 128:(col + 1) * 128],
                                    w38[:, th, :, :],
                                    start=(th == 0), stop=(th == 1),
                                    perf_mode=DRM, skip_group_check=True)
                        nc.scalar.activation(logits[:, b, :], ps3[:], AF.Copy)

            # ------------------------------------- softplus + masked sums
            # softplus(t) = relu(t) - ln(sigmoid(|t|)); |t| is sign-agnostic
            # so Abs/Sigmoid/Ln run over all 21 blocks at once.
            tc.tile_set_cur_wait(MLP0 + (NBLK + 4) * BLK_MS)
            sp_a = cp.tile([128, NBLK, 8], f32, tag="sp_a")
            sp_r = cp.tile([128, NBLK, 8], f32, tag="sp_r")
            sp = cp.tile([128, NBLK, 8], f32, tag="sp")
            spn_sum = cp.tile([128, 8], f32, tag="spn_sum")
            junk = cp.tile([128, 3, 8], f32, tag="junk")
            vcol = cp.tile([128, 4], f32, tag="vcol")
            nc.scalar.activation(sp_a[:], logits[:], AF.Abs, bias=b3f)
            nc.scalar.activation(sp_r[:, :NEG, :], logits[:, :NEG, :],
                                 AF.Relu, bias=b3f)
            nc.scalar.activation(sp_r[:, NEG, :], logits[:, NEG, :],
                                 AF.Relu, bias=-b3f, scale=-1.0)
            nc.scalar.activation(sp_a[:], sp_a[:], AF.Sigmoid)
            nc.scalar.activation(sp_a[:], sp_a[:], AF.Ln)
            nc.vector.tensor_sub(sp[:], sp_r[:], sp_a[:])
            nc.vector.tensor_reduce(
                spn_sum[:], sp[:, :NEG, :].transpose([0, 2, 1]),
                mybir.AxisListType.X, ALU.add)
            nc.any.memset(vcol[:], 0.0)
            nc.vector.scalar_tensor_tensor(
                junk[:, 0, :], in0=sp[:, NEG, :], scalar=1.0, in1=mfT[:],
                op0=ALU.mult, op1=ALU.mult, accum_out=vcol[:, 0:1])
            nc.vector.scalar_tensor_tensor(
                junk[:, 1, :], in0=spn_sum[:], scalar=1.0, in1=mfT[:],
                op0=ALU.mult, op1=ALU.mult, accum_out=vcol[:, 1:2])
            nc.vector.tensor_scalar(
                junk[:, 2, :], mfT[:], 1.0, 0.0, op0=ALU.mult,
                op1=ALU.add, accum_out=vcol[:, 2:3])

            ones = cp.tile([128, 1], f32, tag="ones")
            nc.any.memset(ones[:], 1.0)
            psf = pa_pool.tile([128, 8], f32, tag="pa3")
            nc.tensor.matmul(psf[0:1, 0:4], ones[:], vcol[:],
                             start=True, stop=True)
            out_sb = cp.tile([1, 4], f32, tag="out_sb")
            nc.scalar.activation(out_sb[:], psf[0:1, 0:4], AF.Copy)
            nc.sync.dma_start(out=d_out[:], in_=out_sb[:])

    nc.finalize()
    return nc


def _get_program(u_list, k_eff, b3f):
    key = (tuple(u_list), k_eff, float(b3f))
    if key not in _PROGRAM_CACHE:
        _PROGRAM_CACHE[key] = _build_program(u_list, k_eff, b3f)
    return _PROGRAM_CACHE[key]


# ------------------------------------------------------------------ kernel

def kernel(**inputs):
    u_list = [int(x) for x in np.asarray(inputs["unroll_subsample"]).reshape(-1)]
    k_eff = max(u_list) + 1
    shared = _prep_shared(inputs, u_list, k_eff)
    nc = _get_program(u_list, k_eff, shared["b3f"])

    wmaps = {k: v for k, v in shared.items()
             if k in ("w_hh8", "w1a8", "w1b8", "w28", "w38", "bhhg",
                      "b1T", "b2T")}
    in_maps = []
    for c in range(NC):
        m = dict(wmaps)
        m.update(_prep_core(c, inputs, shared, u_list, k_eff))
        in_maps.append(m)

    res = bass_utils.run_bass_kernel_spmd(nc, in_maps, list(range(NC)))
    P = Ng = D = 0.0
    for c in range(NC):
        o = np.asarray(res.results[c]["out"], np.float64)
        P += o[0, 0]
        Ng += o[0, 1]
        D += o[0, 2]
    loss = COEFF * (P / D + Ng / (D * NEG))
    return np.float32(loss)
